# revision 1
# baseline (speedup 1.0000x reference)
"""Causal self-attention with RoPE, tensor-parallel over heads on 8 trn2 cores.

Reference computation (B=1, T=4096, C=1024, h=16, d=64, fp32):
    q/k/v = x @ W{q,k,v}^T ; rope(q), rope(k) ; causal softmax(q k^T / 8) v ; @ Wo^T

Sharding: 2 heads per core (tensor parallel). Each core reads the full x
(transposed + bf16 on host) and its slice of Wq/Wk/Wv (column-parallel) and
Wo (row-parallel). Cores emit partial o-projections; the host sums them.

Device-side layout choices:
  - qT/kT [dhead(=128 both heads) x T] with the head dim de-interleaved
    (rope real parts in partitions 0-31 / 64-95, imag in 32-63 / 96-127) so
    rope's pair-swap is a partition-block swap done by 4 small DMAs.
  - scores are computed transposed: sT[j, i] = sum_d kT[d,j] qT[d,i], so the
    softmax normalizer is a sum over PARTITIONS, obtained for free by
    augmenting v with a ones column in the att @ v matmul (row 64 of the
    y-psum accumulates the denominator).
  - v is produced transposed like q/k then PE-transposed to natural [t, d]
    blocks (needed as the stationary operand of the att@v matmul).
  - causal masking: only diagonal j-tiles need masking; 4 static [128,512]
    masks (one per 128-offset within a 512 column group) multiply exp'd
    scores. Fully-masked subtiles are skipped in the att@v accumulation.
"""

import numpy as np
import ml_dtypes

bf16 = ml_dtypes.bfloat16

T, C, H, D = 4096, 1024, 16, 64
NCORES = 8
HPC = H // NCORES          # heads per core
DD = HPC * D               # per-core qkv features (=128)
P = 128

_nc_cache = {}


def _build_nc(t=T):
    import concourse.bass as bass
    import concourse.tile as tile
    import concourse.mybir as mybir
    from concourse import bacc
    from concourse.masks import make_identity

    f32 = mybir.dt.float32
    b16 = mybir.dt.bfloat16
    MUL = mybir.AluOpType.mult
    EXP = mybir.ActivationFunctionType.Exp

    nt = t // 512            # qkv t-chunks
    nw = t // 1024           # attention query windows
    njb = t // P             # key blocks

    nc = bacc.Bacc("TRN2")

    xt_d = nc.dram_tensor("xt", [C, t], b16, kind="ExternalInput")
    wq_d = nc.dram_tensor("wq", [C, DD], b16, kind="ExternalInput")
    wk_d = nc.dram_tensor("wk", [C, DD], b16, kind="ExternalInput")
    wv_d = nc.dram_tensor("wv", [C, DD], b16, kind="ExternalInput")
    wo_d = nc.dram_tensor("wo", [DD, C], b16, kind="ExternalInput")
    cos_d = nc.dram_tensor("cosb", [P, t], b16, kind="ExternalInput")
    sin_d = nc.dram_tensor("sinb", [P, t], b16, kind="ExternalInput")
    msk_d = nc.dram_tensor("mask4", [P, 4, 512], b16, kind="ExternalInput")
    out_d = nc.dram_tensor("opart", [t, C], f32, kind="ExternalOutput")

    with tile.TileContext(nc) as tc:
        with (
            tc.tile_pool(name="const", bufs=1) as constp,
            tc.tile_pool(name="xload", bufs=3) as xload,
            tc.tile_pool(name="rope", bufs=3) as ropep,
            tc.tile_pool(name="att", bufs=4) as attp,
            tc.tile_pool(name="small", bufs=4) as smallp,
        ):
            # ---- constants / persistent tensors (weights first: the first
            # matmuls need them; cos/sin before first rope; wo/mask later) ----
            wq_sb = constp.tile([P, C // P, DD], b16)
            nc.sync.dma_start(wq_sb, wq_d[:].rearrange("(co p) m -> p co m", p=P))
            wk_sb = constp.tile([P, C // P, DD], b16)
            nc.sync.dma_start(wk_sb, wk_d[:].rearrange("(co p) m -> p co m", p=P))
            wv_sb = constp.tile([P, C // P, DD], b16)
            nc.sync.dma_start(wv_sb, wv_d[:].rearrange("(co p) m -> p co m", p=P))
            cos_sb = constp.tile([P, t], b16)
            nc.sync.dma_start(cos_sb, cos_d[:])
            sin_sb = constp.tile([P, t], b16)
            nc.sync.dma_start(sin_sb, sin_d[:])
            ident = constp.tile([P, P], b16)
            make_identity(nc, ident)
            msk_sb = constp.tile([P, 4, 512], b16)
            nc.sync.dma_start(msk_sb, msk_d[:])

            qT = constp.tile([P, t], b16)   # rope'd q, both heads
            kT = constp.tile([P, t], b16)
            yT = constp.tile([P, t], b16)   # normalized attention output
            # v in natural layout per 128-block, +ones cols at 64 and 129
            vaug = constp.tile([P, njb, 2 * D + 2], b16)
            nc.vector.memset(vaug[:, :, D], 1.0)
            nc.vector.memset(vaug[:, :, 2 * D + 1], 1.0)

            # ---- phase 1: qkv projections + rope + v transpose,
            # with the first two 512-wide attention windows interleaved so
            # the ACT engine starts exp work while qkv is still streaming.
            # PSUM: ph1 drains 4 banks (bufs=1) + early-attention 4 banks.
            with (
                tc.tile_pool(name="psqkv", bufs=1, space="PSUM") as psqkv,
                tc.tile_pool(name="psearly", bufs=1, space="PSUM") as psearly,
            ):
                vts = {}

                def v_transposes(tch):
                    vt = vts.pop(tch)
                    for tb in range(4):
                        pst = psqkv.tile([P, P], b16, tag="pst", name="pst")
                        nc.tensor.transpose(pst, vt[:, tb * P:(tb + 1) * P], ident)
                        g = tch * 4 + tb
                        nc.vector.tensor_copy(vaug[:, g, 0:D], pst[:, 0:D])
                        nc.vector.tensor_copy(vaug[:, g, D + 1:2 * D + 1],
                                              pst[:, D:2 * D])

                def qkv_chunk(tch):
                    tsl = slice(tch * 512, (tch + 1) * 512)
                    xt = xload.tile([P, C // P, 512], b16, name="xt")
                    nc.sync.dma_start(
                        xt, xt_d[:].rearrange("(co p) t -> p co t", p=P)[:, :, tsl]
                    )
                    pss_qkv = {}
                    for name, w_sb in (("q", wq_sb), ("k", wk_sb), ("v", wv_sb)):
                        ps = psqkv.tile([P, 512], f32, tag=f"ps_{name}",
                                        name=f"ps_{name}")
                        for ci in range(C // P):
                            nc.tensor.matmul(
                                ps, w_sb[:, ci], xt[:, ci],
                                start=(ci == 0), stop=(ci == C // P - 1),
                            )
                        pss_qkv[name] = ps
                    if tch > 0:
                        v_transposes(tch - 1)
                    qks = {}
                    for name in ("q", "k"):
                        qf = ropep.tile([P, 512], b16, tag=f"qf_{name}",
                                        name="qf")
                        nc.vector.tensor_copy(qf, pss_qkv[name])
                        sw = ropep.tile([P, 512], b16, tag=f"sw_{name}",
                                        name="sw")
                        nc.sync.dma_start(sw[0:32], qf[32:64])
                        nc.sync.dma_start(sw[32:64], qf[0:32])
                        nc.sync.dma_start(sw[64:96], qf[96:128])
                        nc.sync.dma_start(sw[96:128], qf[64:96])
                        qks[name] = (qf, sw)
                    t1s = {}
                    for name in ("q", "k"):
                        t1 = ropep.tile([P, 512], b16, tag=f"t1_{name}",
                                        name="t1")
                        nc.vector.tensor_tensor(t1, qks[name][0],
                                                cos_sb[:, tsl], MUL)
                        t1s[name] = t1
                    t2s = {}
                    for name in ("q", "k"):
                        t2 = ropep.tile([P, 512], b16, tag=f"t2_{name}",
                                        name="t2")
                        nc.vector.tensor_tensor(t2, qks[name][1],
                                                sin_sb[:, tsl], MUL)
                        t2s[name] = t2
                    for name, dest in (("q", qT), ("k", kT)):
                        nc.vector.tensor_add(dest[:, tsl], t1s[name], t2s[name])
                    vt = ropep.tile([P, 512], b16, tag="vt", name="vt")
                    nc.vector.tensor_copy(vt, pss_qkv["v"])
                    vts[tch] = vt

                def early_window(iw):
                    # W=512 attention window over i in [512*iw, 512*iw+512)
                    psyE = {}
                    for h in range(HPC):
                        psyE[h] = psearly.tile([D + 1, 512], f32,
                                               tag=f"psyE{h}", name="psyE")
                    isl = slice(iw * 512, (iw + 1) * 512)
                    njc = 4 * (iw + 1)
                    for jc in range(njc):
                        for h in range(HPC):
                            hb = D * h
                            jsl = slice(jc * P, (jc + 1) * P)
                            pssE = psearly.tile([P, 512], f32, tag=f"pssE{h}",
                                                name="pssE")
                            nc.tensor.matmul(pssE, kT[hb:hb + D, jsl],
                                             qT[hb:hb + D, isl],
                                             start=True, stop=True)
                            attE = attp.tile([P, 512], b16, tag=f"attE{h}",
                                             name="attE")
                            nc.scalar.activation(attE, pssE, EXP, scale=0.125)
                            if jc >= 4 * iw:
                                nc.vector.tensor_tensor(
                                    attE, attE, msk_sb[:, jc - 4 * iw], MUL)
                            va = vaug[:, jc, (D + 1) * h:(D + 1) * h + D + 1]
                            nc.tensor.matmul(psyE[h], va, attE,
                                             start=(jc == 0),
                                             stop=(jc == njc - 1))
                    for h in range(HPC):
                        rec = smallp.tile([1, 512], f32, tag="rec", name="rec")
                        nc.vector.reciprocal(rec, psyE[h][D:D + 1, :])
                        recb = smallp.tile([D, 512], f32, tag="recb",
                                           name="recb")
                        nc.gpsimd.partition_broadcast(recb, rec)
                        nc.vector.tensor_tensor(yT[D * h:D * h + D, isl],
                                                psyE[h][0:D, :], recb, MUL)

                qkv_chunk(0)
                if nt > 1:
                    qkv_chunk(1)          # emits v_transposes(0)
                early_window(0)           # needs vaug blocks 0..3
                if nt > 2:
                    qkv_chunk(2)          # emits v_transposes(1)
                else:
                    v_transposes(1)
                for tch in range(3, min(4, nt)):
                    qkv_chunk(tch)
                early_window(1)           # needs vaug blocks 0..7
                for tch in range(4, nt):
                    qkv_chunk(tch)
                for tch in sorted(vts):
                    v_transposes(tch)

            # load wo while attention starts (not needed until o_proj)
            wo_sb = constp.tile([DD, C], b16)
            nc.sync.dma_start(wo_sb, wo_d[:])

            # ---- phase 2: attention, 1024-wide query windows ----
            # scoresT[j,i] per (head, jc); exp on ACT (psum->sbuf, scale=1/8);
            # diagonal tiles masked; att@v accumulates y + denominator (ones
            # column of vaug). As soon as a sub-window's accumulation is done
            # (sub0 at jc=8*icg+3), it is normalized and its o-projection is
            # emitted, reusing the freed psy bank slots -- this overlaps the
            # boundary work with the rest of the window.
            with tc.tile_pool(name="psatt", bufs=1, space="PSUM") as psatt:
                def norm_and_oproj(icg, sub, psys):
                    for h in range(HPC):
                        isl = slice(icg * 1024 + sub * 512,
                                    icg * 1024 + sub * 512 + 512)
                        rec = smallp.tile([1, 512], f32, tag="rec")
                        nc.vector.reciprocal(rec, psys[h, sub][D:D + 1, :])
                        recb = smallp.tile([D, 512], f32, tag="recb")
                        nc.gpsimd.partition_broadcast(recb, rec)
                        nc.vector.tensor_tensor(
                            yT[D * h:D * h + D, isl],
                            psys[h, sub][0:D, :], recb, MUL,
                        )
                    for tb in range(icg * 8 + sub * 4, icg * 8 + sub * 4 + 4):
                        for mc in range(C // 512):
                            pso = psatt.tile([P, 512], f32,
                                             tag=f"psy{tb % 2}{sub}",
                                             name="pso")
                            nc.tensor.matmul(
                                pso, yT[:, tb * P:(tb + 1) * P],
                                wo_sb[:, mc * 512:(mc + 1) * 512],
                                start=True, stop=True,
                            )
                            ob = attp.tile([P, 512], f32, tag="ob")
                            nc.vector.tensor_copy(ob, pso)
                            nc.sync.dma_start(
                                out_d[tb * P:(tb + 1) * P,
                                      mc * 512:(mc + 1) * 512],
                                ob,
                            )

                # o-projection for the early windows' rows (t 0:1024)
                for tb in range(8):
                    for mc in range(C // 512):
                        pso = psatt.tile([P, 512], f32,
                                         tag=f"psy{tb % 2}{tb // 4}",
                                         name="pso")
                        nc.tensor.matmul(
                            pso, yT[:, tb * P:(tb + 1) * P],
                            wo_sb[:, mc * 512:(mc + 1) * 512],
                            start=True, stop=True,
                        )
                        ob = attp.tile([P, 512], f32, tag="ob", name="ob")
                        nc.vector.tensor_copy(ob, pso)
                        nc.sync.dma_start(
                            out_d[tb * P:(tb + 1) * P,
                                  mc * 512:(mc + 1) * 512],
                            ob,
                        )

                for icg in range(1, nw):
                    psys = {}
                    for h in range(HPC):
                        for sub in range(2):
                            psys[h, sub] = psatt.tile(
                                [D + 1, 512], f32, tag=f"psy{h}{sub}",
                                name=f"psy{h}{sub}",
                            )
                    njc = 8 * icg + 8
                    for jc in range(njc):
                        for h in range(HPC):
                            hb = D * h
                            jsl = slice(jc * P, (jc + 1) * P)
                            pss = psatt.tile([P, 1024], f32, tag=f"pss{h}",
                                             name="pss")
                            for sub in range(2):
                                isl = slice(icg * 1024 + sub * 512,
                                            icg * 1024 + sub * 512 + 512)
                                nc.tensor.matmul(
                                    pss[:, sub * 512:(sub + 1) * 512],
                                    kT[hb:hb + D, jsl], qT[hb:hb + D, isl],
                                    start=True, stop=True,
                                )
                            diag = jc >= 8 * icg
                            s0 = 512 * ((jc - 8 * icg) // 4) if diag else 0
                            att = attp.tile([P, 1024], b16, tag=f"att{h}",
                                            name="att")
                            nc.scalar.activation(att[:, s0:], pss[:, s0:], EXP,
                                                 scale=0.125)
                            if diag:
                                m = jc % 4
                                nc.vector.tensor_tensor(
                                    att[:, s0:s0 + 512], att[:, s0:s0 + 512],
                                    msk_sb[:, m], MUL,
                                )
                            for sub in range(2):
                                last_jc = 8 * icg + 4 * (sub + 1) - 1
                                if jc > last_jc:
                                    continue
                                va = vaug[:, jc, (D + 1) * h:(D + 1) * h + D + 1]
                                nc.tensor.matmul(
                                    psys[h, sub][:],
                                    va, att[:, sub * 512:(sub + 1) * 512],
                                    start=(jc == 0), stop=(jc == last_jc),
                                )
                        if jc == 8 * icg + 3:
                            norm_and_oproj(icg, 0, psys)
                    norm_and_oproj(icg, 1, psys)

    nc.compile()
    return nc


def _perm_deinterleave():
    """Row permutation for Wq/Wk: per head, even rows then odd rows."""
    perm = []
    for h in range(H):
        base = h * D
        perm += [base + 2 * k for k in range(D // 2)]
        perm += [base + 2 * k + 1 for k in range(D // 2)]
    return np.array(perm)


def make_core_inputs(x, freqs_cos, freqs_sin, Wq, Wk, Wv, Wo, t=T):
    """Host-side sharding/layout prep. Returns per-core input dicts."""
    x = np.asarray(x, np.float32).reshape(t, C)
    fc = np.asarray(freqs_cos, np.float32)
    fs = np.asarray(freqs_sin, np.float32)
    Wq = np.asarray(Wq, np.float32)
    Wk = np.asarray(Wk, np.float32)
    Wv = np.asarray(Wv, np.float32)
    Wo = np.asarray(Wo, np.float32)

    xt = np.ascontiguousarray(x.T).astype(bf16)                  # [C, t]
    perm = _perm_deinterleave()
    Wq_p, Wk_p = Wq[perm], Wk[perm]

    # rope factor tables in the de-interleaved [dd, t] layout
    kidx = np.arange(P) % 32
    sgn = np.where((np.arange(P) // 32) % 2 == 0, -1.0, 1.0).astype(np.float32)
    cosb = fc.T[kidx].astype(bf16)                               # [128, t]
    sinb = (fs.T[kidx] * sgn[:, None]).astype(bf16)

    # diagonal-tile causal masks: mask4[j, m, i] = 1 iff 128*m + j <= i
    jj = np.arange(P)[:, None, None]
    mm = np.arange(4)[None, :, None]
    ii = np.arange(512)[None, None, :]
    mask4 = ((P * mm + jj) <= ii).astype(bf16)

    in_maps = []
    for c in range(NCORES):
        rows = slice(c * DD, (c + 1) * DD)
        in_maps.append({
            "xt": xt,
            "wq": np.ascontiguousarray(Wq_p[rows].T).astype(bf16),
            "wk": np.ascontiguousarray(Wk_p[rows].T).astype(bf16),
            "wv": np.ascontiguousarray(Wv[rows].T).astype(bf16),
            "wo": np.ascontiguousarray(Wo[:, rows].T).astype(bf16),
            "cosb": cosb,
            "sinb": sinb,
            "mask4": mask4,
        })
    return in_maps


def run(inputs, trace=False):
    """Compile once, run on 8 cores, host-sum partials. Returns (out, results)."""
    import sys
    if "/opt/trn_rl_repo" not in sys.path:
        sys.path.insert(0, "/opt/trn_rl_repo")
    from concourse.bass_utils import run_bass_kernel_spmd

    if "nc" not in _nc_cache:
        _nc_cache["nc"] = _build_nc()
    nc = _nc_cache["nc"]

    in_maps = make_core_inputs(**inputs)
    res = run_bass_kernel_spmd(nc, in_maps, core_ids=list(range(NCORES)),
                               trace=trace)
    out = np.zeros((T, C), np.float64)
    for r in res.results:
        out += r["opart"].astype(np.float64)
    return out.astype(np.float32).reshape(1, T, C), res


def kernel(**inputs):
    import sys
    if "/opt/trn_rl_repo" not in sys.path:
        sys.path.insert(0, "/opt/trn_rl_repo")
    out, _ = run(inputs)
    return out



# revision 41
# speedup vs baseline: 1.2133x; 1.2133x over previous
"""Causal self-attention with RoPE, tensor-parallel over heads on 8 trn2 cores.

Reference computation (B=1, T=4096, C=1024, h=16, d=64, fp32):
    q/k/v = x @ W{q,k,v}^T ; rope(q), rope(k) ; causal softmax(q k^T / 8) v ; @ Wo^T

Sharding: 2 heads per core (tensor parallel). Each core reads the full x
(transposed + bf16 on host) and its slice of Wq/Wk/Wv (column-parallel) and
Wo (row-parallel). Cores emit partial o-projections; the host sums them.

Device-side layout choices:
  - qT/kT [dhead(=128 both heads) x T] with the head dim de-interleaved
    (rope real parts in partitions 0-31 / 64-95, imag in 32-63 / 96-127) so
    rope's pair-swap is a partition-block swap done by 4 small DMAs.
  - scores are computed transposed: sT[j, i] = sum_d kT[d,j] qT[d,i], so the
    softmax normalizer is a sum over PARTITIONS, obtained for free by
    augmenting v with a ones column in the att @ v matmul (row 64 of the
    y-psum accumulates the denominator).
  - v is produced transposed like q/k then PE-transposed to natural [t, d]
    blocks (needed as the stationary operand of the att@v matmul).
  - causal masking: only diagonal j-tiles need masking; 4 static [128,512]
    masks (one per 128-offset within a 512 column group) multiply exp'd
    scores. Fully-masked subtiles are skipped in the att@v accumulation.
"""

import numpy as np
import ml_dtypes

bf16 = ml_dtypes.bfloat16

T, C, H, D = 4096, 1024, 16, 64
NCORES = 8
HPC = H // NCORES          # heads per core
DD = HPC * D               # per-core qkv features (=128)
P = 128

_nc_cache = {}


def _build_nc(t=T):
    import concourse.bass as bass
    import concourse.tile as tile
    import concourse.mybir as mybir
    from concourse import bacc
    from concourse.masks import make_identity

    f32 = mybir.dt.float32
    b16 = mybir.dt.bfloat16
    MUL = mybir.AluOpType.mult
    EXP = mybir.ActivationFunctionType.Exp

    nt = t // 512            # qkv t-chunks
    nw = t // 1024           # attention query windows
    njb = t // P             # key blocks

    nc = bacc.Bacc("TRN2")

    xt_d = nc.dram_tensor("xt", [C, t], b16, kind="ExternalInput")
    # weights are host-prepacked to [P, C//P, DD] so the load is one
    # contiguous 2KB run per partition (full-rate DMA descriptors)
    wq_d = nc.dram_tensor("wq", [P, C // P, DD], b16, kind="ExternalInput")
    wk_d = nc.dram_tensor("wk", [P, C // P, DD], b16, kind="ExternalInput")
    wv_d = nc.dram_tensor("wv", [P, C // P, DD], b16, kind="ExternalInput")
    wo_d = nc.dram_tensor("wo", [DD, C], b16, kind="ExternalInput")
    cos_d = nc.dram_tensor("cosb", [P, t], b16, kind="ExternalInput")
    sin_d = nc.dram_tensor("sinb", [P, t], b16, kind="ExternalInput")
    msk_d = nc.dram_tensor("mask4", [P, 4, 512], b16, kind="ExternalInput")
    out_d = nc.dram_tensor("opart", [t, C], b16, kind="ExternalOutput")

    with tile.TileContext(nc) as tc:
        with (
            tc.tile_pool(name="const", bufs=1) as constp,
            tc.tile_pool(name="xload", bufs=3) as xload,
            tc.tile_pool(name="rope", bufs=3) as ropep,
            tc.tile_pool(name="att", bufs=4) as attp,
            tc.tile_pool(name="small", bufs=4) as smallp,
            tc.tile_pool(name="ps", bufs=1, space="PSUM") as psp,
        ):
            # ---- constants / persistent tensors. DMA issue order is chosen
            # so the first qkv matmuls (need wq + x chunk 0) start ASAP:
            # wq -> xt0 -> wk/wv -> cos/sin (needed at first rope) -> rest.
            xt_pre = {}

            def load_xt(tch):
                tsl = slice(tch * 512, (tch + 1) * 512)
                xt = xload.tile([P, C // P, 512], b16, name="xt")
                nc.sync.dma_start(
                    xt, xt_d[:].rearrange("(co p) t -> p co t", p=P)[:, :, tsl]
                )
                xt_pre[tch] = xt

            wq_sb = constp.tile([P, C // P, DD], b16)
            nc.sync.dma_start(wq_sb, wq_d[:])
            # x chunk 0 in two halves so the first qkv matmuls start sooner
            xt0 = xload.tile([P, C // P, 512], b16, name="xt0")
            xt_view = xt_d[:].rearrange("(co p) t -> p co t", p=P)
            nc.sync.dma_start(xt0[:, 0:4], xt_view[:, 0:4, 0:512])
            nc.sync.dma_start(xt0[:, 4:8], xt_view[:, 4:8, 0:512])
            xt_pre[0] = xt0
            wk_sb = constp.tile([P, C // P, DD], b16)
            nc.sync.dma_start(wk_sb, wk_d[:])
            load_xt(1)
            wv_sb = constp.tile([P, C // P, DD], b16)
            nc.sync.dma_start(wv_sb, wv_d[:])
            # rope tables: only the first two chunks' columns now — the big
            # tails load after chunk 0/1's swap DMAs so the first rope isn't
            # stuck behind them in the transfer queue
            cos_sb = constp.tile([P, t], b16)
            nc.sync.dma_start(cos_sb[:, 0:1024], cos_d[:, 0:1024])
            sin_sb = constp.tile([P, t], b16)
            nc.sync.dma_start(sin_sb[:, 0:1024], sin_d[:, 0:1024])
            wo_sb = constp.tile([DD, C], b16)
            msk_sb = constp.tile([P, 4, 512], b16)

            def late_consts():
                nc.sync.dma_start(msk_sb, msk_d[:])
                nc.sync.dma_start(cos_sb[:, 1024:], cos_d[:, 1024:])
                nc.sync.dma_start(sin_sb[:, 1024:], sin_d[:, 1024:])
                nc.sync.dma_start(wo_sb, wo_d[:])

            qT = constp.tile([P, t], b16)   # rope'd q, both heads
            kT = constp.tile([P, t], b16)
            yT = constp.tile([P, t], b16)   # normalized attention output
            # v in natural layout per 128-block, +ones cols at 64 and 129
            vaug = constp.tile([P, njb, 2 * D + 2], b16)
            nc.vector.memset(vaug[:, :, D], 1.0)
            nc.vector.memset(vaug[:, :, 2 * D + 1], 1.0)

            # PSUM budget (8 banks): scratch 2KB x2 (qkv psums + o-proj) +
            # pss2 4KB x2 (scores, double-buffered) + psyE{h} 2KB x2 (y+denom
            # accumulators) = 16KB.
            def scratch():
                return psp.tile([P, 512], f32, tag="scratch", bufs=2,
                                name="scr")

            def qkv_parts(tch, cp):
                """Six micro-closures (~0.85us of PE each) so a chunk's
                matmuls can be spread one-per-jc through an attention
                window's PE slack. cp = engine for the psum->sbuf copies
                (ACT before the windows saturate it, DVE after)."""
                tsl = slice(tch * 512, (tch + 1) * 512)
                st = {}

                def copy(dst, src_):
                    if cp is nc.scalar:
                        nc.scalar.copy(dst, src_)
                    else:
                        cp.tensor_copy(dst, src_)

                def mm_half(name, w_sb, half):
                    if name == "q" and half == 0:
                        st["xt"] = xt_pre.pop(tch)
                        st["q"] = scratch()
                    if name == "k" and half == 0:
                        if tch + 2 < nt and tch + 2 not in xt_pre:
                            load_xt(tch + 2)
                    if half == 0 and name != "q":
                        st[name] = scratch()
                    ps = st[name]
                    for ci in range(4 * half, 4 * half + 4):
                        nc.tensor.matmul(
                            ps, w_sb[:, ci], st["xt"][:, ci],
                            start=(ci == 0), stop=(ci == C // P - 1),
                        )

                def q1():
                    mm_half("q", wq_sb, 0)

                def q2():
                    mm_half("q", wq_sb, 1)
                    # q and k share one [128, 2, 512] tile so the rope
                    # pair-swap is 4 partition-block DMAs covering both
                    st["qf2"] = ropep.tile([P, 2, 512], b16, tag="qf2",
                                           name="qf2")
                    copy(st["qf2"][:, 0], st["q"])

                def k1():
                    mm_half("k", wk_sb, 0)

                def k2():
                    mm_half("k", wk_sb, 1)
                    qf2 = st["qf2"]
                    copy(qf2[:, 1], st["k"])
                    sw2 = ropep.tile([P, 2, 512], b16, tag="sw2", name="sw2")
                    nc.sync.dma_start(sw2[0:32], qf2[32:64])
                    nc.sync.dma_start(sw2[32:64], qf2[0:32])
                    nc.sync.dma_start(sw2[64:96], qf2[96:128])
                    nc.sync.dma_start(sw2[96:128], qf2[64:96])
                    st["sw2"] = sw2
                    for i, name in enumerate(("q", "k")):
                        tl = ropep.tile([P, 512], b16, tag=f"t1_{name}",
                                        name="t1")
                        nc.vector.tensor_tensor(tl, qf2[:, i],
                                                cos_sb[:, tsl], MUL)
                        st[f"t1{name}"] = tl

                def v1():
                    t2q = ropep.tile([P, 512], b16, tag="t2_q", name="t2")
                    nc.vector.tensor_tensor(t2q, st["sw2"][:, 0],
                                            sin_sb[:, tsl], MUL)
                    nc.vector.tensor_add(qT[:, tsl], st["t1q"], t2q)
                    mm_half("v", wv_sb, 0)

                def v2():
                    mm_half("v", wv_sb, 1)
                    t2k = ropep.tile([P, 512], b16, tag="t2_k", name="t2")
                    nc.vector.tensor_tensor(t2k, st["sw2"][:, 1],
                                            sin_sb[:, tsl], MUL)
                    nc.vector.tensor_add(kT[:, tsl], st["t1k"], t2k)
                    # v: psum->sbuf copy, XBAR dma-transpose to a dense
                    # [t, d] tile (strided destinations come out scrambled),
                    # then two narrow copies into vaug around the ones column
                    vt = ropep.tile([P, 512], b16, tag="vt", name="vt")
                    copy(vt, st["v"])
                    dense = ropep.tile([P, 4, P], b16, tag="vdense",
                                       name="vdense")
                    nc.sync.dma_start_transpose(dense, vt[:])
                    g0 = tch * 4
                    nc.vector.tensor_copy(vaug[:, g0:g0 + 4, 0:D],
                                          dense[:, :, 0:D])
                    nc.vector.tensor_copy(vaug[:, g0:g0 + 4, D + 1:2 * D + 1],
                                          dense[:, :, D:2 * D])

                return [q1, q2, k1, k2, v1, v2]

            def qkv_chunk(tch):
                for f in qkv_parts(tch, nc.scalar):
                    f()


            def oproj_fillers(iw, tail=False):
                """o-projection of rows [512iw, 512iw+512) as 4 closures of
                2 matmuls each. psum->sbuf copies alternate DVE/GPSIMD (plus
                ACT in the tail, once exp traffic is done)."""
                outs = []
                for tb in range(4 * iw, 4 * iw + 4):
                    def one(tb=tb):
                        for mc in range(C // 512):
                            pso = scratch()
                            nc.tensor.matmul(
                                pso, yT[:, tb * P:(tb + 1) * P],
                                wo_sb[:, mc * 512:(mc + 1) * 512],
                                start=True, stop=True,
                            )
                            ob = attp.tile([P, 512], b16, tag="ob",
                                           name="ob")
                            # GPSIMD cannot read PSUM; DVE in-window, with
                            # ACT helping once the exp stream is done
                            if tail and (2 * tb + mc) % 2 == 1:
                                nc.scalar.copy(ob, pso)
                            else:
                                nc.vector.tensor_copy(ob, pso)
                            nc.sync.dma_start(
                                out_d[tb * P:(tb + 1) * P,
                                      mc * 512:(mc + 1) * 512],
                                ob,
                            )
                    outs.append(one)
                return outs

            def win512(iw, parts=(), oproj=(), last=False):
                """512-wide attention window over i in [512iw, 512iw+512),
                one-jc software-pipelined, with ONE merged exp per jc
                covering both heads ([128, 2x512]). Fillers run in the PE
                slack of the ACT-bound jc loop: qkv chunk micro-parts from
                jc>=1 (always ready), the previous window's o-projection
                from jc>=3 (needs its normalize to have drained). Returns
                this window's o-projection closures for the next window."""
                isl = slice(iw * 512, (iw + 1) * 512)
                njc = 4 * iw + 4
                parts = list(parts)
                oproj = list(oproj)
                psyE = {}
                for h in range(HPC):
                    psyE[h] = psp.tile([D + 1, 512], f32, tag=f"psyE{h}",
                                       name="psyE")

                def emit_attv(p):
                    jc0, a2 = p
                    for h in range(HPC):
                        va = vaug[:, jc0, (D + 1) * h:(D + 1) * h + D + 1]
                        nc.tensor.matmul(psyE[h], va, a2[:, h],
                                         start=(jc0 == 0),
                                         stop=(jc0 == njc - 1))

                pend = []
                for jc in range(njc):
                    jsl = slice(jc * P, (jc + 1) * P)
                    ps2 = psp.tile([P, 2, 512], f32, tag="pss2", bufs=2,
                                   name="ps2")
                    for h in range(HPC):
                        hb = D * h
                        nc.tensor.matmul(ps2[:, h], kT[hb:hb + D, jsl],
                                         qT[hb:hb + D, isl],
                                         start=True, stop=True)
                    att2 = attp.tile([P, 2, 512], b16, tag="att2",
                                     name="att2")
                    nc.scalar.activation(att2, ps2, EXP, scale=0.125)
                    if jc >= 4 * iw:
                        m = jc - 4 * iw
                        for h in range(HPC):
                            nc.vector.tensor_tensor(att2[:, h], att2[:, h],
                                                    msk_sb[:, m], MUL)
                    # 2-deep attv pipeline: the first att@v lands at jc2, by
                    # which time the previous window's normalize (the WAR on
                    # this window's psyE accumulators) has drained
                    if len(pend) == 2:
                        emit_attv(pend.pop(0))
                    pend.append((jc, att2))
                    if jc < njc - 1:
                        if jc >= 1 and parts:
                            parts.pop(0)()
                        elif jc >= 3 and oproj:
                            oproj.pop(0)()
                for p in pend:
                    emit_attv(p)
                for f in parts + oproj:   # window too small for its fillers
                    f()
                recs, recbs = {}, {}
                for h in range(HPC):
                    recs[h] = smallp.tile([1, 512], f32, tag=f"rec{h}",
                                          name="rec")
                    nc.vector.reciprocal(recs[h], psyE[h][D:D + 1, :])
                for h in range(HPC):
                    recbs[h] = smallp.tile([D, 512], f32, tag=f"recb{h}",
                                           name="recb")
                    nc.gpsimd.partition_broadcast(recbs[h], recs[h])
                for h in range(HPC):
                    nc.vector.tensor_tensor(yT[D * h:D * h + D, isl],
                                            psyE[h][0:D, :], recbs[h], MUL)
                return oproj_fillers(iw, tail=last)

            qkv_chunk(0)
            c1 = qkv_parts(1, nc.scalar)
            for f in c1[:4]:
                f()
            late_consts()
            op = win512(0, parts=c1[4:] + qkv_parts(2, nc.vector))
            op = win512(1, parts=qkv_parts(3, nc.vector), oproj=op)
            op = win512(2, parts=qkv_parts(4, nc.vector), oproj=op)
            op = win512(3, parts=qkv_parts(5, nc.vector), oproj=op)
            op = win512(4, parts=qkv_parts(6, nc.vector), oproj=op)
            op = win512(5, parts=qkv_parts(7, nc.vector), oproj=op)
            op = win512(6, oproj=op)
            op = win512(7, oproj=op, last=True)
            for f in op:
                f()

    nc.compile()
    return nc


def _perm_deinterleave():
    """Row permutation for Wq/Wk: per head, even rows then odd rows."""
    perm = []
    for h in range(H):
        base = h * D
        perm += [base + 2 * k for k in range(D // 2)]
        perm += [base + 2 * k + 1 for k in range(D // 2)]
    return np.array(perm)


def make_core_inputs(x, freqs_cos, freqs_sin, Wq, Wk, Wv, Wo, t=T):
    """Host-side sharding/layout prep. Returns per-core input dicts."""
    x = np.asarray(x, np.float32).reshape(t, C)
    fc = np.asarray(freqs_cos, np.float32)
    fs = np.asarray(freqs_sin, np.float32)
    Wq = np.asarray(Wq, np.float32)
    Wk = np.asarray(Wk, np.float32)
    Wv = np.asarray(Wv, np.float32)
    Wo = np.asarray(Wo, np.float32)

    xt = np.ascontiguousarray(x.T).astype(bf16)                  # [C, t]
    perm = _perm_deinterleave()
    Wq_p, Wk_p = Wq[perm], Wk[perm]

    # rope factor tables in the de-interleaved [dd, t] layout
    kidx = np.arange(P) % 32
    sgn = np.where((np.arange(P) // 32) % 2 == 0, -1.0, 1.0).astype(np.float32)
    cosb = fc.T[kidx].astype(bf16)                               # [128, t]
    sinb = (fs.T[kidx] * sgn[:, None]).astype(bf16)

    # diagonal-tile causal masks: mask4[j, m, i] = 1 iff 128*m + j <= i
    jj = np.arange(P)[:, None, None]
    mm = np.arange(4)[None, :, None]
    ii = np.arange(512)[None, None, :]
    mask4 = ((P * mm + jj) <= ii).astype(bf16)

    def pack_w(w):
        # [C, DD] -> [P, C//P, DD]: one contiguous 2KB run per partition
        return np.ascontiguousarray(
            w.T.reshape(C // P, P, DD).transpose(1, 0, 2)).astype(bf16)

    in_maps = []
    for c in range(NCORES):
        rows = slice(c * DD, (c + 1) * DD)
        in_maps.append({
            "xt": xt,
            "wq": pack_w(Wq_p[rows]),
            "wk": pack_w(Wk_p[rows]),
            "wv": pack_w(Wv[rows]),
            "wo": np.ascontiguousarray(Wo[:, rows].T).astype(bf16),
            "cosb": cosb,
            "sinb": sinb,
            "mask4": mask4,
        })
    return in_maps


def run(inputs, trace=False):
    """Compile once, run on 8 cores, host-sum partials. Returns (out, results)."""
    import sys
    if "/opt/trn_rl_repo" not in sys.path:
        sys.path.insert(0, "/opt/trn_rl_repo")
    from concourse.bass_utils import run_bass_kernel_spmd

    if "nc" not in _nc_cache:
        _nc_cache["nc"] = _build_nc()
    nc = _nc_cache["nc"]

    in_maps = make_core_inputs(**inputs)
    res = run_bass_kernel_spmd(nc, in_maps, core_ids=list(range(NCORES)),
                               trace=trace)
    out = np.zeros((T, C), np.float64)
    for r in res.results:
        out += r["opart"].astype(np.float64)
    return out.astype(np.float32).reshape(1, T, C), res


def kernel(**inputs):
    import sys
    if "/opt/trn_rl_repo" not in sys.path:
        sys.path.insert(0, "/opt/trn_rl_repo")
    out, _ = run(inputs)
    return out



# revision 45
# speedup vs baseline: 1.2992x; 1.0708x over previous
"""Causal self-attention with RoPE, tensor-parallel over heads on 8 trn2 cores.

Reference computation (B=1, T=4096, C=1024, h=16, d=64, fp32):
    q/k/v = x @ W{q,k,v}^T ; rope(q), rope(k) ; causal softmax(q k^T / 8) v ; @ Wo^T

Sharding: 2 heads per core (tensor parallel). Each core reads the full x
(transposed + bf16 on host) and its slice of Wq/Wk/Wv (column-parallel) and
Wo (row-parallel). Cores emit partial o-projections; the host sums them.

Device-side layout choices:
  - qT/kT [dhead(=128 both heads) x T] with the head dim de-interleaved
    (rope real parts in partitions 0-31 / 64-95, imag in 32-63 / 96-127) so
    rope's pair-swap is a partition-block swap done by 4 small DMAs.
  - scores are computed transposed: sT[j, i] = sum_d kT[d,j] qT[d,i], so the
    softmax normalizer is a sum over PARTITIONS, obtained for free by
    augmenting v with a ones column in the att @ v matmul (row 64 of the
    y-psum accumulates the denominator).
  - v is produced transposed like q/k then PE-transposed to natural [t, d]
    blocks (needed as the stationary operand of the att@v matmul).
  - causal masking: only diagonal j-tiles need masking; 4 static [128,512]
    masks (one per 128-offset within a 512 column group) multiply exp'd
    scores. Fully-masked subtiles are skipped in the att@v accumulation.
"""

import numpy as np
import ml_dtypes

bf16 = ml_dtypes.bfloat16

T, C, H, D = 4096, 1024, 16, 64
NCORES = 8
HPC = H // NCORES          # heads per core
DD = HPC * D               # per-core qkv features (=128)
P = 128

_nc_cache = {}


def _build_nc(t=T):
    import concourse.bass as bass
    import concourse.tile as tile
    import concourse.mybir as mybir
    from concourse import bacc
    from concourse.masks import make_identity

    f32 = mybir.dt.float32
    b16 = mybir.dt.bfloat16
    MUL = mybir.AluOpType.mult
    EXP = mybir.ActivationFunctionType.Exp

    nt = t // 512            # qkv t-chunks
    nw = t // 1024           # attention query windows
    njb = t // P             # key blocks

    nc = bacc.Bacc("TRN2")

    xt_d = nc.dram_tensor("xt", [C, t], b16, kind="ExternalInput")
    # weights are host-prepacked to [P, C//P, DD] so the load is one
    # contiguous 2KB run per partition (full-rate DMA descriptors)
    wq_d = nc.dram_tensor("wq", [P, C // P, DD], b16, kind="ExternalInput")
    wk_d = nc.dram_tensor("wk", [P, C // P, DD], b16, kind="ExternalInput")
    wv_d = nc.dram_tensor("wv", [P, C // P, DD], b16, kind="ExternalInput")
    wo_d = nc.dram_tensor("wo", [DD, C], b16, kind="ExternalInput")
    cos_d = nc.dram_tensor("cosb", [P, t], b16, kind="ExternalInput")
    sin_d = nc.dram_tensor("sinb", [P, t], b16, kind="ExternalInput")
    msk_d = nc.dram_tensor("mask4", [P, 4, 512], b16, kind="ExternalInput")
    out_d = nc.dram_tensor("opart", [t, C], b16, kind="ExternalOutput")

    with tile.TileContext(nc) as tc:
        with (
            tc.tile_pool(name="const", bufs=1) as constp,
            tc.tile_pool(name="xload", bufs=3) as xload,
            tc.tile_pool(name="rope", bufs=3) as ropep,
            tc.tile_pool(name="att", bufs=4) as attp,
            tc.tile_pool(name="small", bufs=4) as smallp,
            tc.tile_pool(name="ps", bufs=1, space="PSUM") as psp,
        ):
            # ---- constants / persistent tensors. DMA issue order is chosen
            # so the first qkv matmuls (need wq + x chunk 0) start ASAP:
            # wq -> xt0 -> wk/wv -> cos/sin (needed at first rope) -> rest.
            xt_pre = {}

            def load_xt(tch):
                tsl = slice(tch * 512, (tch + 1) * 512)
                xt = xload.tile([P, C // P, 512], b16, name="xt")
                nc.sync.dma_start(
                    xt, xt_d[:].rearrange("(co p) t -> p co t", p=P)[:, :, tsl]
                )
                xt_pre[tch] = xt

            wq_sb = constp.tile([P, C // P, DD], b16)
            nc.sync.dma_start(wq_sb, wq_d[:])
            # x chunk 0 in four slices so the first qkv matmuls start sooner
            xt0 = xload.tile([P, C // P, 512], b16, name="xt0")
            xt_view = xt_d[:].rearrange("(co p) t -> p co t", p=P)
            for c4 in range(4):
                nc.sync.dma_start(xt0[:, 2 * c4:2 * c4 + 2],
                                  xt_view[:, 2 * c4:2 * c4 + 2, 0:512])
            xt_pre[0] = xt0
            wk_sb = constp.tile([P, C // P, DD], b16)
            nc.sync.dma_start(wk_sb, wk_d[:])
            load_xt(1)
            wv_sb = constp.tile([P, C // P, DD], b16)
            nc.sync.dma_start(wv_sb, wv_d[:])
            # rope tables: only the first two chunks' columns now — the big
            # tails load after chunk 0/1's swap DMAs so the first rope isn't
            # stuck behind them in the transfer queue
            cos_sb = constp.tile([P, t], b16)
            nc.sync.dma_start(cos_sb[:, 0:1024], cos_d[:, 0:1024])
            sin_sb = constp.tile([P, t], b16)
            nc.sync.dma_start(sin_sb[:, 0:1024], sin_d[:, 0:1024])
            wo_sb = constp.tile([DD, C], b16)
            msk_sb = constp.tile([P, 4, 512], b16)

            def late_consts():
                nc.sync.dma_start(msk_sb, msk_d[:])
                nc.sync.dma_start(cos_sb[:, 1024:], cos_d[:, 1024:])
                nc.sync.dma_start(sin_sb[:, 1024:], sin_d[:, 1024:])
                nc.sync.dma_start(wo_sb, wo_d[:])

            qT = constp.tile([P, t], b16)   # rope'd q, both heads
            kT = constp.tile([P, t], b16)
            yT = constp.tile([P, t], b16)   # normalized attention output
            # v in natural layout per 128-block, +ones cols at 64 and 129
            vaug = constp.tile([P, njb, 2 * D + 2], b16)
            nc.vector.memset(vaug[:, :, D], 1.0)
            nc.vector.memset(vaug[:, :, 2 * D + 1], 1.0)

            # PSUM budget (8 banks): scratch 2KB x2 (qkv psums + o-proj) +
            # pss2 4KB x2 (scores, double-buffered) + psyE{h} 2KB x2 (y+denom
            # accumulators) = 16KB.
            def scratch():
                return psp.tile([P, 512], f32, tag="scratch", bufs=2,
                                name="scr")

            def qkv_parts(tch, cp):
                """Six micro-closures (~0.85us of PE each) so a chunk's
                matmuls can be spread one-per-jc through an attention
                window's PE slack. cp = engine for the psum->sbuf copies
                (ACT before the windows saturate it, DVE after)."""
                tsl = slice(tch * 512, (tch + 1) * 512)
                st = {}

                def copy(dst, src_):
                    if cp is nc.scalar:
                        nc.scalar.copy(dst, src_)
                    else:
                        cp.tensor_copy(dst, src_)

                def mm_half(name, w_sb, half):
                    if name == "q" and half == 0:
                        st["xt"] = xt_pre.pop(tch)
                        st["q"] = scratch()
                    if name == "k" and half == 0:
                        if tch + 2 < nt and tch + 2 not in xt_pre:
                            load_xt(tch + 2)
                    if half == 0 and name != "q":
                        st[name] = scratch()
                    ps = st[name]
                    for ci in range(4 * half, 4 * half + 4):
                        nc.tensor.matmul(
                            ps, w_sb[:, ci], st["xt"][:, ci],
                            start=(ci == 0), stop=(ci == C // P - 1),
                        )

                def q1():
                    mm_half("q", wq_sb, 0)

                def q2():
                    mm_half("q", wq_sb, 1)
                    # q and k share one [128, 2, 512] tile so the rope
                    # pair-swap is 4 partition-block DMAs covering both
                    st["qf2"] = ropep.tile([P, 2, 512], b16, tag="qf2",
                                           name="qf2")
                    copy(st["qf2"][:, 0], st["q"])

                def k1():
                    mm_half("k", wk_sb, 0)

                def k2():
                    mm_half("k", wk_sb, 1)
                    qf2 = st["qf2"]
                    copy(qf2[:, 1], st["k"])
                    sw2 = ropep.tile([P, 2, 512], b16, tag="sw2", name="sw2")
                    nc.sync.dma_start(sw2[0:32], qf2[32:64])
                    nc.sync.dma_start(sw2[32:64], qf2[0:32])
                    nc.sync.dma_start(sw2[64:96], qf2[96:128])
                    nc.sync.dma_start(sw2[96:128], qf2[64:96])
                    st["sw2"] = sw2
                    for i, name in enumerate(("q", "k")):
                        tl = ropep.tile([P, 512], b16, tag=f"t1_{name}",
                                        name="t1")
                        nc.vector.tensor_tensor(tl, qf2[:, i],
                                                cos_sb[:, tsl], MUL)
                        st[f"t1{name}"] = tl

                def v1():
                    t2q = ropep.tile([P, 512], b16, tag="t2_q", name="t2")
                    nc.vector.tensor_tensor(t2q, st["sw2"][:, 0],
                                            sin_sb[:, tsl], MUL)
                    nc.vector.tensor_add(qT[:, tsl], st["t1q"], t2q)
                    mm_half("v", wv_sb, 0)

                def v2():
                    mm_half("v", wv_sb, 1)
                    t2k = ropep.tile([P, 512], b16, tag="t2_k", name="t2")
                    nc.vector.tensor_tensor(t2k, st["sw2"][:, 1],
                                            sin_sb[:, tsl], MUL)
                    nc.vector.tensor_add(kT[:, tsl], st["t1k"], t2k)
                    # v: psum->sbuf copy, XBAR dma-transpose to a dense
                    # [t, d] tile (strided destinations come out scrambled),
                    # then two narrow copies into vaug around the ones column
                    vt = ropep.tile([P, 512], b16, tag="vt", name="vt")
                    copy(vt, st["v"])
                    dense = ropep.tile([P, 4, P], b16, tag="vdense",
                                       name="vdense")
                    nc.sync.dma_start_transpose(dense, vt[:])
                    g0 = tch * 4
                    nc.vector.tensor_copy(vaug[:, g0:g0 + 4, 0:D],
                                          dense[:, :, 0:D])
                    nc.vector.tensor_copy(vaug[:, g0:g0 + 4, D + 1:2 * D + 1],
                                          dense[:, :, D:2 * D])

                return [q1, q2, k1, k2, v1, v2]

            def qkv_chunk(tch):
                for f in qkv_parts(tch, nc.scalar):
                    f()


            def oproj_fillers(iw, tail=False):
                """o-projection of rows [512iw, 512iw+512) as 4 closures of
                2 matmuls each. psum->sbuf copies alternate DVE/GPSIMD (plus
                ACT in the tail, once exp traffic is done)."""
                outs = []
                for tb in range(4 * iw, 4 * iw + 4):
                    def one(tb=tb):
                        for mc in range(C // 512):
                            if tail and (2 * tb + mc) % 2 == 1:
                                # the score double-buffers are free once the
                                # last window's exps are done: 4-slot rotation
                                pso = psp.tile([P, 512], f32, tag="pss2",
                                               bufs=2, name="pso2")
                            else:
                                pso = scratch()
                            nc.tensor.matmul(
                                pso, yT[:, tb * P:(tb + 1) * P],
                                wo_sb[:, mc * 512:(mc + 1) * 512],
                                start=True, stop=True,
                            )
                            ob = attp.tile([P, 512], b16, tag="ob",
                                           name="ob")
                            # GPSIMD cannot read PSUM; DVE in-window, with
                            # ACT helping once the exp stream is done
                            if tail and (2 * tb + mc) % 2 == 1:
                                nc.scalar.copy(ob, pso)
                            else:
                                nc.vector.tensor_copy(ob, pso)
                            nc.sync.dma_start(
                                out_d[tb * P:(tb + 1) * P,
                                      mc * 512:(mc + 1) * 512],
                                ob,
                            )
                    outs.append(one)
                return outs

            def win512(iw, parts=(), oproj=(), last=False):
                """512-wide attention window over i in [512iw, 512iw+512),
                one-jc software-pipelined, with ONE merged exp per jc
                covering both heads ([128, 2x512]). Fillers run in the PE
                slack of the ACT-bound jc loop: qkv chunk micro-parts from
                jc>=1 (always ready), the previous window's o-projection
                from jc>=3 (needs its normalize to have drained). Returns
                this window's o-projection closures for the next window."""
                isl = slice(iw * 512, (iw + 1) * 512)
                njc = 4 * iw + 4
                parts = list(parts)
                oproj = list(oproj)
                psyE = {}
                for h in range(HPC):
                    psyE[h] = psp.tile([D + 1, 512], f32, tag=f"psyE{h}",
                                       name="psyE")

                def emit_attv(p):
                    jc0, a2 = p
                    for h in range(HPC):
                        va = vaug[:, jc0, (D + 1) * h:(D + 1) * h + D + 1]
                        nc.tensor.matmul(psyE[h], va, a2[:, h],
                                         start=(jc0 == 0),
                                         stop=(jc0 == njc - 1))

                pend = []
                for jc in range(njc):
                    jsl = slice(jc * P, (jc + 1) * P)
                    ps2 = psp.tile([P, 2, 512], f32, tag="pss2", bufs=2,
                                   name="ps2")
                    for h in range(HPC):
                        hb = D * h
                        nc.tensor.matmul(ps2[:, h], kT[hb:hb + D, jsl],
                                         qT[hb:hb + D, isl],
                                         start=True, stop=True)
                    att2 = attp.tile([P, 2, 512], b16, tag="att2",
                                     name="att2")
                    nc.scalar.activation(att2, ps2, EXP, scale=0.125)
                    if jc >= 4 * iw:
                        m = jc - 4 * iw
                        for h in range(HPC):
                            nc.vector.tensor_tensor(att2[:, h], att2[:, h],
                                                    msk_sb[:, m], MUL)
                    # 2-deep attv pipeline: the first att@v lands at jc2, by
                    # which time the previous window's normalize (the WAR on
                    # this window's psyE accumulators) has drained
                    if len(pend) == 3:
                        emit_attv(pend.pop(0))
                    pend.append((jc, att2))
                    if jc < njc - 1:
                        if jc >= 1 and parts:
                            parts.pop(0)()
                        elif jc >= 3 and jc % 2 == 1 and oproj:
                            oproj.pop(0)()
                for p in pend:
                    emit_attv(p)
                for f in parts + oproj:   # window too small for its fillers
                    f()
                recs, recbs = {}, {}
                for h in range(HPC):
                    recs[h] = smallp.tile([1, 512], f32, tag=f"rec{h}",
                                          name="rec")
                    nc.vector.reciprocal(recs[h], psyE[h][D:D + 1, :])
                for h in range(HPC):
                    recbs[h] = smallp.tile([D, 512], f32, tag=f"recb{h}",
                                           name="recb")
                    nc.gpsimd.partition_broadcast(recbs[h], recs[h])
                for h in range(HPC):
                    nc.vector.tensor_tensor(yT[D * h:D * h + D, isl],
                                            psyE[h][0:D, :], recbs[h], MUL)
                return oproj_fillers(iw, tail=last)

            qkv_chunk(0)
            c1 = qkv_parts(1, nc.scalar)
            for f in c1[:4]:
                f()
            late_consts()
            op0 = win512(0, parts=c1[4:] + qkv_parts(2, nc.scalar))
            op1 = win512(1, parts=qkv_parts(3, nc.scalar))
            op2 = win512(2, parts=qkv_parts(4, nc.scalar), oproj=op0)
            op3 = win512(3, parts=qkv_parts(5, nc.vector), oproj=op1)
            op4 = win512(4, parts=qkv_parts(6, nc.vector), oproj=op2)
            op5 = win512(5, parts=qkv_parts(7, nc.vector), oproj=op3)
            op6 = win512(6, oproj=op4 + op5)
            op7 = win512(7, oproj=op6, last=True)
            for f in op7:
                f()

    nc.compile()
    return nc


def _perm_deinterleave():
    """Row permutation for Wq/Wk: per head, even rows then odd rows."""
    perm = []
    for h in range(H):
        base = h * D
        perm += [base + 2 * k for k in range(D // 2)]
        perm += [base + 2 * k + 1 for k in range(D // 2)]
    return np.array(perm)


def make_core_inputs(x, freqs_cos, freqs_sin, Wq, Wk, Wv, Wo, t=T):
    """Host-side sharding/layout prep. Returns per-core input dicts."""
    x = np.asarray(x, np.float32).reshape(t, C)
    fc = np.asarray(freqs_cos, np.float32)
    fs = np.asarray(freqs_sin, np.float32)
    Wq = np.asarray(Wq, np.float32)
    Wk = np.asarray(Wk, np.float32)
    Wv = np.asarray(Wv, np.float32)
    Wo = np.asarray(Wo, np.float32)

    xt = np.ascontiguousarray(x.T).astype(bf16)                  # [C, t]
    perm = _perm_deinterleave()
    Wq_p, Wk_p = Wq[perm], Wk[perm]

    # rope factor tables in the de-interleaved [dd, t] layout
    kidx = np.arange(P) % 32
    sgn = np.where((np.arange(P) // 32) % 2 == 0, -1.0, 1.0).astype(np.float32)
    cosb = fc.T[kidx].astype(bf16)                               # [128, t]
    sinb = (fs.T[kidx] * sgn[:, None]).astype(bf16)

    # diagonal-tile causal masks: mask4[j, m, i] = 1 iff 128*m + j <= i
    jj = np.arange(P)[:, None, None]
    mm = np.arange(4)[None, :, None]
    ii = np.arange(512)[None, None, :]
    mask4 = ((P * mm + jj) <= ii).astype(bf16)

    def pack_w(w):
        # [C, DD] -> [P, C//P, DD]: one contiguous 2KB run per partition
        return np.ascontiguousarray(
            w.T.reshape(C // P, P, DD).transpose(1, 0, 2)).astype(bf16)

    in_maps = []
    for c in range(NCORES):
        rows = slice(c * DD, (c + 1) * DD)
        in_maps.append({
            "xt": xt,
            "wq": pack_w(Wq_p[rows]),
            "wk": pack_w(Wk_p[rows]),
            "wv": pack_w(Wv[rows]),
            "wo": np.ascontiguousarray(Wo[:, rows].T).astype(bf16),
            "cosb": cosb,
            "sinb": sinb,
            "mask4": mask4,
        })
    return in_maps


def run(inputs, trace=False):
    """Compile once, run on 8 cores, host-sum partials. Returns (out, results)."""
    import sys
    if "/opt/trn_rl_repo" not in sys.path:
        sys.path.insert(0, "/opt/trn_rl_repo")
    from concourse.bass_utils import run_bass_kernel_spmd

    if "nc" not in _nc_cache:
        _nc_cache["nc"] = _build_nc()
    nc = _nc_cache["nc"]

    in_maps = make_core_inputs(**inputs)
    res = run_bass_kernel_spmd(nc, in_maps, core_ids=list(range(NCORES)),
                               trace=trace)
    out = np.zeros((T, C), np.float64)
    for r in res.results:
        out += r["opart"].astype(np.float64)
    return out.astype(np.float32).reshape(1, T, C), res


def kernel(**inputs):
    import sys
    if "/opt/trn_rl_repo" not in sys.path:
        sys.path.insert(0, "/opt/trn_rl_repo")
    out, _ = run(inputs)
    return out



# revision 46
# speedup vs baseline: 1.3366x; 1.0288x over previous
"""Causal self-attention with RoPE, tensor-parallel over heads on 8 trn2 cores.

Reference computation (B=1, T=4096, C=1024, h=16, d=64, fp32):
    q/k/v = x @ W{q,k,v}^T ; rope(q), rope(k) ; causal softmax(q k^T / 8) v ; @ Wo^T

Sharding: 2 heads per core (tensor parallel). Each core reads the full x
(transposed + bf16 on host) and its slice of Wq/Wk/Wv (column-parallel) and
Wo (row-parallel). Cores emit partial o-projections; the host sums them.

Device-side layout choices:
  - qT/kT [dhead(=128 both heads) x T] with the head dim de-interleaved
    (rope real parts in partitions 0-31 / 64-95, imag in 32-63 / 96-127) so
    rope's pair-swap is a partition-block swap done by 4 small DMAs.
  - scores are computed transposed: sT[j, i] = sum_d kT[d,j] qT[d,i], so the
    softmax normalizer is a sum over PARTITIONS, obtained for free by
    augmenting v with a ones column in the att @ v matmul (row 64 of the
    y-psum accumulates the denominator).
  - v is produced transposed like q/k then PE-transposed to natural [t, d]
    blocks (needed as the stationary operand of the att@v matmul).
  - causal masking: only diagonal j-tiles need masking; 4 static [128,512]
    masks (one per 128-offset within a 512 column group) multiply exp'd
    scores. Fully-masked subtiles are skipped in the att@v accumulation.
"""

import numpy as np
import ml_dtypes

bf16 = ml_dtypes.bfloat16

T, C, H, D = 4096, 1024, 16, 64
NCORES = 8
HPC = H // NCORES          # heads per core
DD = HPC * D               # per-core qkv features (=128)
P = 128

_nc_cache = {}


def _build_nc(t=T):
    import concourse.bass as bass
    import concourse.tile as tile
    import concourse.mybir as mybir
    from concourse import bacc
    from concourse.masks import make_identity

    f32 = mybir.dt.float32
    b16 = mybir.dt.bfloat16
    MUL = mybir.AluOpType.mult
    EXP = mybir.ActivationFunctionType.Exp

    nt = t // 512            # qkv t-chunks
    nw = t // 1024           # attention query windows
    njb = t // P             # key blocks

    nc = bacc.Bacc("TRN2")

    xt_d = nc.dram_tensor("xt", [C, t], b16, kind="ExternalInput")
    # weights are host-prepacked to [P, C//P, DD] so the load is one
    # contiguous 2KB run per partition (full-rate DMA descriptors)
    wq_d = nc.dram_tensor("wq", [P, C // P, DD], b16, kind="ExternalInput")
    wk_d = nc.dram_tensor("wk", [P, C // P, DD], b16, kind="ExternalInput")
    wv_d = nc.dram_tensor("wv", [P, C // P, DD], b16, kind="ExternalInput")
    wo_d = nc.dram_tensor("wo", [DD, C], b16, kind="ExternalInput")
    cos_d = nc.dram_tensor("cosb", [P, t], b16, kind="ExternalInput")
    sin_d = nc.dram_tensor("sinb", [P, t], b16, kind="ExternalInput")
    msk_d = nc.dram_tensor("mask4", [P, 4, 512], b16, kind="ExternalInput")
    out_d = nc.dram_tensor("opart", [t, C], b16, kind="ExternalOutput")

    with tile.TileContext(nc) as tc:
        with (
            tc.tile_pool(name="const", bufs=1) as constp,
            tc.tile_pool(name="xload", bufs=3) as xload,
            tc.tile_pool(name="rope", bufs=3) as ropep,
            tc.tile_pool(name="att", bufs=4) as attp,
            tc.tile_pool(name="small", bufs=4) as smallp,
            tc.tile_pool(name="ps", bufs=1, space="PSUM") as psp,
        ):
            # ---- constants / persistent tensors. DMA issue order is chosen
            # so the first qkv matmuls (need wq + x chunk 0) start ASAP:
            # wq -> xt0 -> wk/wv -> cos/sin (needed at first rope) -> rest.
            xt_pre = {}

            def load_xt(tch):
                tsl = slice(tch * 512, (tch + 1) * 512)
                xt = xload.tile([P, C // P, 512], b16, name="xt")
                nc.sync.dma_start(
                    xt, xt_d[:].rearrange("(co p) t -> p co t", p=P)[:, :, tsl]
                )
                xt_pre[tch] = xt

            wq_sb = constp.tile([P, C // P, DD], b16)
            nc.sync.dma_start(wq_sb, wq_d[:])
            # x chunk 0 in four slices so the first qkv matmuls start sooner
            xt0 = xload.tile([P, C // P, 512], b16, name="xt0")
            xt_view = xt_d[:].rearrange("(co p) t -> p co t", p=P)
            for c4 in range(4):
                nc.sync.dma_start(xt0[:, 2 * c4:2 * c4 + 2],
                                  xt_view[:, 2 * c4:2 * c4 + 2, 0:512])
            xt_pre[0] = xt0
            wk_sb = constp.tile([P, C // P, DD], b16)
            nc.sync.dma_start(wk_sb, wk_d[:])
            load_xt(1)
            wv_sb = constp.tile([P, C // P, DD], b16)
            nc.sync.dma_start(wv_sb, wv_d[:])
            # rope tables: only the first two chunks' columns now — the big
            # tails load after chunk 0/1's swap DMAs so the first rope isn't
            # stuck behind them in the transfer queue
            cos_sb = constp.tile([P, t], b16)
            nc.sync.dma_start(cos_sb[:, 0:1024], cos_d[:, 0:1024])
            sin_sb = constp.tile([P, t], b16)
            nc.sync.dma_start(sin_sb[:, 0:1024], sin_d[:, 0:1024])
            wo_sb = constp.tile([DD, C], b16)
            msk_sb = constp.tile([P, 4, 512], b16)

            def late_consts():
                nc.sync.dma_start(msk_sb, msk_d[:])
                nc.sync.dma_start(cos_sb[:, 1024:], cos_d[:, 1024:])
                nc.sync.dma_start(sin_sb[:, 1024:], sin_d[:, 1024:])
                nc.sync.dma_start(wo_sb, wo_d[:])

            qT = constp.tile([P, t], b16)   # rope'd q, both heads
            kT = constp.tile([P, t], b16)
            yT = constp.tile([P, t], b16)   # normalized attention output
            # v in natural layout per 128-block, +ones cols at 64 and 129
            vaug = constp.tile([P, njb, 2 * D + 2], b16)
            nc.vector.memset(vaug[:, :, D], 1.0)
            nc.vector.memset(vaug[:, :, 2 * D + 1], 1.0)

            # PSUM budget (8 banks): scratch 2KB x2 (qkv psums + o-proj) +
            # pss2 4KB x2 (scores, double-buffered) + psyE{h} 2KB x2 (y+denom
            # accumulators) = 16KB.
            def scratch():
                return psp.tile([P, 512], f32, tag="scratch", bufs=2,
                                name="scr")

            def qkv_parts(tch, cp):
                """Six micro-closures (~0.85us of PE each) so a chunk's
                matmuls can be spread one-per-jc through an attention
                window's PE slack. cp = engine for the psum->sbuf copies
                (ACT before the windows saturate it, DVE after)."""
                tsl = slice(tch * 512, (tch + 1) * 512)
                st = {}

                def copy(dst, src_):
                    if cp is nc.scalar:
                        nc.scalar.copy(dst, src_)
                    else:
                        cp.tensor_copy(dst, src_)

                def mm_half(name, w_sb, half):
                    if name == "q" and half == 0:
                        st["xt"] = xt_pre.pop(tch)
                        st["q"] = scratch()
                    if name == "k" and half == 0:
                        if tch + 2 < nt and tch + 2 not in xt_pre:
                            load_xt(tch + 2)
                    if half == 0 and name != "q":
                        st[name] = scratch()
                    ps = st[name]
                    for ci in range(4 * half, 4 * half + 4):
                        nc.tensor.matmul(
                            ps, w_sb[:, ci], st["xt"][:, ci],
                            start=(ci == 0), stop=(ci == C // P - 1),
                        )

                def q1():
                    mm_half("q", wq_sb, 0)

                def q2():
                    mm_half("q", wq_sb, 1)
                    # q and k share one [128, 2, 512] tile so the rope
                    # pair-swap is 4 partition-block DMAs covering both
                    st["qf2"] = ropep.tile([P, 2, 512], b16, tag="qf2",
                                           name="qf2")
                    copy(st["qf2"][:, 0], st["q"])

                def k1():
                    mm_half("k", wk_sb, 0)

                def k2():
                    mm_half("k", wk_sb, 1)
                    qf2 = st["qf2"]
                    copy(qf2[:, 1], st["k"])
                    sw2 = ropep.tile([P, 2, 512], b16, tag="sw2", name="sw2")
                    nc.sync.dma_start(sw2[0:32], qf2[32:64])
                    nc.sync.dma_start(sw2[32:64], qf2[0:32])
                    nc.sync.dma_start(sw2[64:96], qf2[96:128])
                    nc.sync.dma_start(sw2[96:128], qf2[64:96])
                    st["sw2"] = sw2
                    for i, name in enumerate(("q", "k")):
                        tl = ropep.tile([P, 512], b16, tag=f"t1_{name}",
                                        name="t1")
                        nc.vector.tensor_tensor(tl, qf2[:, i],
                                                cos_sb[:, tsl], MUL)
                        st[f"t1{name}"] = tl

                def v1():
                    t2q = ropep.tile([P, 512], b16, tag="t2_q", name="t2")
                    nc.vector.tensor_tensor(t2q, st["sw2"][:, 0],
                                            sin_sb[:, tsl], MUL)
                    nc.vector.tensor_add(qT[:, tsl], st["t1q"], t2q)
                    mm_half("v", wv_sb, 0)

                def v2():
                    mm_half("v", wv_sb, 1)
                    t2k = ropep.tile([P, 512], b16, tag="t2_k", name="t2")
                    nc.vector.tensor_tensor(t2k, st["sw2"][:, 1],
                                            sin_sb[:, tsl], MUL)
                    nc.vector.tensor_add(kT[:, tsl], st["t1k"], t2k)
                    # v: psum->sbuf copy, XBAR dma-transpose to a dense
                    # [t, d] tile (strided destinations come out scrambled),
                    # then two narrow copies into vaug around the ones column
                    vt = ropep.tile([P, 512], b16, tag="vt", name="vt")
                    copy(vt, st["v"])
                    dense = ropep.tile([P, 4, P], b16, tag="vdense",
                                       name="vdense")
                    nc.sync.dma_start_transpose(dense, vt[:])
                    g0 = tch * 4
                    nc.vector.tensor_copy(vaug[:, g0:g0 + 4, 0:D],
                                          dense[:, :, 0:D])
                    nc.vector.tensor_copy(vaug[:, g0:g0 + 4, D + 1:2 * D + 1],
                                          dense[:, :, D:2 * D])

                return [q1, q2, k1, k2, v1, v2]

            def qkv_chunk(tch):
                for f in qkv_parts(tch, nc.scalar):
                    f()


            def oproj_fillers(iw, tail=False, norm=None):
                """o-projection (rows x Wo^T) for four 128-row blocks, one
                batched output DMA per block. At the tail (last window), each
                closure also carries its own 128-col slice of the normalize
                multiplies so the final chain is as short as possible."""
                outs = []
                for tb in range(4 * iw, 4 * iw + 4):
                    def one(tb=tb):
                        if norm is not None:
                            psyE, recbs = norm
                            off = tb * P - iw * 512
                            for h in range(HPC):
                                nc.vector.tensor_tensor(
                                    yT[D * h:D * h + D, tb * P:(tb + 1) * P],
                                    psyE[h][0:D, off:off + P],
                                    recbs[h][:, off:off + P], MUL)
                        ob2 = attp.tile([P, 2, 512], b16, tag="ob",
                                        name="ob")
                        for mc in range(C // 512):
                            if tail and mc == 1:
                                # score double-buffers are free at the tail
                                pso = psp.tile([P, 512], f32, tag="pss2",
                                               bufs=2, name="pso2")
                            else:
                                pso = scratch()
                            nc.tensor.matmul(
                                pso, yT[:, tb * P:(tb + 1) * P],
                                wo_sb[:, mc * 512:(mc + 1) * 512],
                                start=True, stop=True,
                            )
                            if tail and mc == 1:
                                nc.scalar.copy(ob2[:, mc], pso)
                            else:
                                nc.vector.tensor_copy(ob2[:, mc], pso)
                        nc.sync.dma_start(out_d[tb * P:(tb + 1) * P, :], ob2)
                    outs.append(one)
                return outs

            def win512(iw, parts=(), oproj=(), last=False):
                """512-wide attention window over i in [512iw, 512iw+512),
                one-jc software-pipelined, with ONE merged exp per jc
                covering both heads ([128, 2x512]). Fillers run in the PE
                slack of the ACT-bound jc loop: qkv chunk micro-parts from
                jc>=1 (always ready), the previous window's o-projection
                from jc>=3 (needs its normalize to have drained). Returns
                this window's o-projection closures for the next window."""
                isl = slice(iw * 512, (iw + 1) * 512)
                njc = 4 * iw + 4
                parts = list(parts)
                oproj = list(oproj)
                psyE = {}
                for h in range(HPC):
                    psyE[h] = psp.tile([D + 1, 512], f32, tag=f"psyE{h}",
                                       name="psyE")

                def emit_attv(p):
                    jc0, a2 = p
                    for h in range(HPC):
                        va = vaug[:, jc0, (D + 1) * h:(D + 1) * h + D + 1]
                        nc.tensor.matmul(psyE[h], va, a2[:, h],
                                         start=(jc0 == 0),
                                         stop=(jc0 == njc - 1))

                pend = []
                for jc in range(njc):
                    jsl = slice(jc * P, (jc + 1) * P)
                    ps2 = psp.tile([P, 2, 512], f32, tag="pss2", bufs=2,
                                   name="ps2")
                    for h in range(HPC):
                        hb = D * h
                        nc.tensor.matmul(ps2[:, h], kT[hb:hb + D, jsl],
                                         qT[hb:hb + D, isl],
                                         start=True, stop=True)
                    att2 = attp.tile([P, 2, 512], b16, tag="att2",
                                     name="att2")
                    nc.scalar.activation(att2, ps2, EXP, scale=0.125)
                    if jc >= 4 * iw:
                        m = jc - 4 * iw
                        for h in range(HPC):
                            nc.vector.tensor_tensor(att2[:, h], att2[:, h],
                                                    msk_sb[:, m], MUL)
                    # 2-deep attv pipeline: the first att@v lands at jc2, by
                    # which time the previous window's normalize (the WAR on
                    # this window's psyE accumulators) has drained
                    if len(pend) == 3:
                        emit_attv(pend.pop(0))
                    pend.append((jc, att2))
                    if jc < njc - 1:
                        if jc >= 1 and parts:
                            parts.pop(0)()
                        elif jc >= 3 and jc % 2 == 1 and oproj:
                            oproj.pop(0)()
                for p in pend:
                    emit_attv(p)
                for f in parts + oproj:   # window too small for its fillers
                    f()
                recs, recbs = {}, {}
                for h in range(HPC):
                    recs[h] = smallp.tile([1, 512], f32, tag=f"rec{h}",
                                          name="rec")
                    nc.vector.reciprocal(recs[h], psyE[h][D:D + 1, :])
                for h in range(HPC):
                    recbs[h] = smallp.tile([D, 512], f32, tag=f"recb{h}",
                                           name="recb")
                    nc.gpsimd.partition_broadcast(recbs[h], recs[h])
                if last:
                    return oproj_fillers(iw, tail=True, norm=(psyE, recbs))
                for h in range(HPC):
                    nc.vector.tensor_tensor(yT[D * h:D * h + D, isl],
                                            psyE[h][0:D, :], recbs[h], MUL)
                return oproj_fillers(iw)

            qkv_chunk(0)
            c1 = qkv_parts(1, nc.scalar)
            for f in c1[:4]:
                f()
            late_consts()
            op0 = win512(0, parts=c1[4:] + qkv_parts(2, nc.scalar))
            op1 = win512(1, parts=qkv_parts(3, nc.scalar))
            op2 = win512(2, parts=qkv_parts(4, nc.scalar), oproj=op0)
            op3 = win512(3, parts=qkv_parts(5, nc.vector), oproj=op1)
            op4 = win512(4, parts=qkv_parts(6, nc.vector), oproj=op2)
            op5 = win512(5, parts=qkv_parts(7, nc.vector), oproj=op3)
            op6 = win512(6, oproj=op4 + op5)
            op7 = win512(7, oproj=op6, last=True)
            for f in op7:
                f()

    nc.compile()
    return nc


def _perm_deinterleave():
    """Row permutation for Wq/Wk: per head, even rows then odd rows."""
    perm = []
    for h in range(H):
        base = h * D
        perm += [base + 2 * k for k in range(D // 2)]
        perm += [base + 2 * k + 1 for k in range(D // 2)]
    return np.array(perm)


def make_core_inputs(x, freqs_cos, freqs_sin, Wq, Wk, Wv, Wo, t=T):
    """Host-side sharding/layout prep. Returns per-core input dicts."""
    x = np.asarray(x, np.float32).reshape(t, C)
    fc = np.asarray(freqs_cos, np.float32)
    fs = np.asarray(freqs_sin, np.float32)
    Wq = np.asarray(Wq, np.float32)
    Wk = np.asarray(Wk, np.float32)
    Wv = np.asarray(Wv, np.float32)
    Wo = np.asarray(Wo, np.float32)

    xt = np.ascontiguousarray(x.T).astype(bf16)                  # [C, t]
    perm = _perm_deinterleave()
    Wq_p, Wk_p = Wq[perm], Wk[perm]

    # rope factor tables in the de-interleaved [dd, t] layout
    kidx = np.arange(P) % 32
    sgn = np.where((np.arange(P) // 32) % 2 == 0, -1.0, 1.0).astype(np.float32)
    cosb = fc.T[kidx].astype(bf16)                               # [128, t]
    sinb = (fs.T[kidx] * sgn[:, None]).astype(bf16)

    # diagonal-tile causal masks: mask4[j, m, i] = 1 iff 128*m + j <= i
    jj = np.arange(P)[:, None, None]
    mm = np.arange(4)[None, :, None]
    ii = np.arange(512)[None, None, :]
    mask4 = ((P * mm + jj) <= ii).astype(bf16)

    def pack_w(w):
        # [C, DD] -> [P, C//P, DD]: one contiguous 2KB run per partition
        return np.ascontiguousarray(
            w.T.reshape(C // P, P, DD).transpose(1, 0, 2)).astype(bf16)

    in_maps = []
    for c in range(NCORES):
        rows = slice(c * DD, (c + 1) * DD)
        in_maps.append({
            "xt": xt,
            "wq": pack_w(Wq_p[rows]),
            "wk": pack_w(Wk_p[rows]),
            "wv": pack_w(Wv[rows]),
            "wo": np.ascontiguousarray(Wo[:, rows].T).astype(bf16),
            "cosb": cosb,
            "sinb": sinb,
            "mask4": mask4,
        })
    return in_maps


def run(inputs, trace=False):
    """Compile once, run on 8 cores, host-sum partials. Returns (out, results)."""
    import sys
    if "/opt/trn_rl_repo" not in sys.path:
        sys.path.insert(0, "/opt/trn_rl_repo")
    from concourse.bass_utils import run_bass_kernel_spmd

    if "nc" not in _nc_cache:
        _nc_cache["nc"] = _build_nc()
    nc = _nc_cache["nc"]

    in_maps = make_core_inputs(**inputs)
    res = run_bass_kernel_spmd(nc, in_maps, core_ids=list(range(NCORES)),
                               trace=trace)
    out = np.zeros((T, C), np.float64)
    for r in res.results:
        out += r["opart"].astype(np.float64)
    return out.astype(np.float32).reshape(1, T, C), res


def kernel(**inputs):
    import sys
    if "/opt/trn_rl_repo" not in sys.path:
        sys.path.insert(0, "/opt/trn_rl_repo")
    out, _ = run(inputs)
    return out



# revision 54
# speedup vs baseline: 1.3407x; 1.0031x over previous
"""Causal self-attention with RoPE, tensor-parallel over heads on 8 trn2 cores.

Reference computation (B=1, T=4096, C=1024, h=16, d=64, fp32):
    q/k/v = x @ W{q,k,v}^T ; rope(q), rope(k) ; causal softmax(q k^T / 8) v ; @ Wo^T

Sharding: 2 heads per core (tensor parallel). Each core reads the full x
(transposed + bf16 on host) and its slice of Wq/Wk/Wv (column-parallel) and
Wo (row-parallel). Cores emit partial o-projections; the host sums them.

Device-side layout choices:
  - qT/kT [dhead(=128 both heads) x T] with the head dim de-interleaved
    (rope real parts in partitions 0-31 / 64-95, imag in 32-63 / 96-127) so
    rope's pair-swap is a partition-block swap done by 4 small DMAs.
  - scores are computed transposed: sT[j, i] = sum_d kT[d,j] qT[d,i], so the
    softmax normalizer is a sum over PARTITIONS, obtained for free by
    augmenting v with a ones column in the att @ v matmul (row 64 of the
    y-psum accumulates the denominator).
  - v is produced transposed like q/k then PE-transposed to natural [t, d]
    blocks (needed as the stationary operand of the att@v matmul).
  - causal masking: only diagonal j-tiles need masking; 4 static [128,512]
    masks (one per 128-offset within a 512 column group) multiply exp'd
    scores. Fully-masked subtiles are skipped in the att@v accumulation.
"""

import numpy as np
import ml_dtypes

bf16 = ml_dtypes.bfloat16

T, C, H, D = 4096, 1024, 16, 64
NCORES = 8
HPC = H // NCORES          # heads per core
DD = HPC * D               # per-core qkv features (=128)
P = 128

_nc_cache = {}


def _build_nc(t=T):
    import concourse.bass as bass
    import concourse.tile as tile
    import concourse.mybir as mybir
    from concourse import bacc
    from concourse.masks import make_identity

    f32 = mybir.dt.float32
    b16 = mybir.dt.bfloat16
    MUL = mybir.AluOpType.mult
    EXP = mybir.ActivationFunctionType.Exp

    nt = t // 512            # qkv t-chunks
    nw = t // 1024           # attention query windows
    njb = t // P             # key blocks

    nc = bacc.Bacc("TRN2")

    xt_d = nc.dram_tensor("xt", [C, t], b16, kind="ExternalInput")
    # weights are host-prepacked to [P, C//P, DD] so the load is one
    # contiguous 2KB run per partition (full-rate DMA descriptors)
    wq_d = nc.dram_tensor("wq", [P, C // P, DD], b16, kind="ExternalInput")
    wk_d = nc.dram_tensor("wk", [P, C // P, DD], b16, kind="ExternalInput")
    wv_d = nc.dram_tensor("wv", [P, C // P, DD], b16, kind="ExternalInput")
    wo_d = nc.dram_tensor("wo", [DD, C], b16, kind="ExternalInput")
    cos_d = nc.dram_tensor("cosb", [P, t], b16, kind="ExternalInput")
    sin_d = nc.dram_tensor("sinb", [P, t], b16, kind="ExternalInput")
    msk_d = nc.dram_tensor("mask4", [P, 4, 512], b16, kind="ExternalInput")
    out_d = nc.dram_tensor("opart", [t, C], b16, kind="ExternalOutput")

    with tile.TileContext(nc) as tc:
        with (
            tc.tile_pool(name="const", bufs=1) as constp,
            tc.tile_pool(name="xload", bufs=3) as xload,
            tc.tile_pool(name="rope", bufs=3) as ropep,
            tc.tile_pool(name="att", bufs=4) as attp,
            tc.tile_pool(name="small", bufs=4) as smallp,
            tc.tile_pool(name="ps", bufs=1, space="PSUM") as psp,
        ):
            # ---- constants / persistent tensors. DMA issue order is chosen
            # so the first qkv matmuls (need wq + x chunk 0) start ASAP:
            # wq -> xt0 -> wk/wv -> cos/sin (needed at first rope) -> rest.
            xt_pre = {}

            def load_xt(tch):
                tsl = slice(tch * 512, (tch + 1) * 512)
                xt = xload.tile([P, C // P, 512], b16, name="xt")
                nc.sync.dma_start(
                    xt, xt_d[:].rearrange("(co p) t -> p co t", p=P)[:, :, tsl]
                )
                xt_pre[tch] = xt

            wq_sb = constp.tile([P, C // P, DD], b16)
            nc.sync.dma_start(wq_sb, wq_d[:])
            # x chunk 0 in four slices so the first qkv matmuls start sooner
            xt0 = xload.tile([P, C // P, 512], b16, name="xt0")
            xt_view = xt_d[:].rearrange("(co p) t -> p co t", p=P)
            for c4 in range(4):
                nc.sync.dma_start(xt0[:, 2 * c4:2 * c4 + 2],
                                  xt_view[:, 2 * c4:2 * c4 + 2, 0:512])
            xt_pre[0] = xt0
            wk_sb = constp.tile([P, C // P, DD], b16)
            nc.sync.dma_start(wk_sb, wk_d[:])
            load_xt(1)
            wv_sb = constp.tile([P, C // P, DD], b16)
            nc.sync.dma_start(wv_sb, wv_d[:])
            # rope tables: only the first two chunks' columns now — the big
            # tails load after chunk 0/1's swap DMAs so the first rope isn't
            # stuck behind them in the transfer queue
            cos_sb = constp.tile([P, t], b16)
            nc.sync.dma_start(cos_sb[:, 0:1024], cos_d[:, 0:1024])
            sin_sb = constp.tile([P, t], b16)
            nc.sync.dma_start(sin_sb[:, 0:1024], sin_d[:, 0:1024])
            wo_sb = constp.tile([DD, C], b16)
            msk_sb = constp.tile([P, 4, 512], b16)

            def late_consts():
                nc.sync.dma_start(msk_sb, msk_d[:])
                nc.sync.dma_start(cos_sb[:, 1024:], cos_d[:, 1024:])
                nc.sync.dma_start(sin_sb[:, 1024:], sin_d[:, 1024:])
                nc.sync.dma_start(wo_sb, wo_d[:])

            qT = constp.tile([P, t], b16)   # rope'd q, both heads
            kT = constp.tile([P, t], b16)
            yT = constp.tile([P, t], b16)   # normalized attention output
            # v in natural layout per 128-block, +ones cols at 64 and 129
            vaug = constp.tile([P, njb, 2 * D + 2], b16)
            nc.vector.memset(vaug[:, :, D], 1.0)
            nc.vector.memset(vaug[:, :, 2 * D + 1], 1.0)

            # PSUM budget (8 banks): scratch 2KB x2 (qkv psums + o-proj) +
            # pss2 4KB x2 (scores, double-buffered) + psyE{h} 2KB x2 (y+denom
            # accumulators) = 16KB.
            def scratch():
                return psp.tile([P, 512], f32, tag="scratch", bufs=2,
                                name="scr")

            def qkv_parts(tch, cp):
                """Six micro-closures (~0.85us of PE each) so a chunk's
                matmuls can be spread one-per-jc through an attention
                window's PE slack. cp = engine for the psum->sbuf copies
                (ACT before the windows saturate it, DVE after)."""
                tsl = slice(tch * 512, (tch + 1) * 512)
                st = {}

                def copy(dst, src_):
                    if cp is nc.scalar:
                        nc.scalar.copy(dst, src_)
                    else:
                        cp.tensor_copy(dst, src_)

                def mm_half(name, w_sb, half):
                    if name == "q" and half == 0:
                        st["xt"] = xt_pre.pop(tch)
                        st["q"] = scratch()
                    if name == "k" and half == 0:
                        if tch + 2 < nt and tch + 2 not in xt_pre:
                            load_xt(tch + 2)
                    if half == 0 and name != "q":
                        st[name] = scratch()
                    ps = st[name]
                    for ci in range(4 * half, 4 * half + 4):
                        nc.tensor.matmul(
                            ps, w_sb[:, ci], st["xt"][:, ci],
                            start=(ci == 0), stop=(ci == C // P - 1),
                        )

                def q1():
                    mm_half("q", wq_sb, 0)

                def q2():
                    mm_half("q", wq_sb, 1)
                    # q and k share one [128, 2, 512] tile so the rope
                    # pair-swap is 4 partition-block DMAs covering both
                    st["qf2"] = ropep.tile([P, 2, 512], b16, tag="qf2",
                                           name="qf2")
                    copy(st["qf2"][:, 0], st["q"])

                def k1():
                    mm_half("k", wk_sb, 0)

                def k2():
                    mm_half("k", wk_sb, 1)
                    qf2 = st["qf2"]
                    copy(qf2[:, 1], st["k"])
                    sw2 = ropep.tile([P, 2, 512], b16, tag="sw2", name="sw2")
                    nc.sync.dma_start(sw2[0:32], qf2[32:64])
                    nc.sync.dma_start(sw2[32:64], qf2[0:32])
                    nc.sync.dma_start(sw2[64:96], qf2[96:128])
                    nc.sync.dma_start(sw2[96:128], qf2[64:96])
                    st["sw2"] = sw2
                    for i, name in enumerate(("q", "k")):
                        tl = ropep.tile([P, 512], b16, tag=f"t1_{name}",
                                        name="t1")
                        nc.vector.tensor_tensor(tl, qf2[:, i],
                                                cos_sb[:, tsl], MUL)
                        st[f"t1{name}"] = tl

                def v1():
                    t2q = ropep.tile([P, 512], b16, tag="t2_q", name="t2")
                    nc.vector.tensor_tensor(t2q, st["sw2"][:, 0],
                                            sin_sb[:, tsl], MUL)
                    nc.vector.tensor_add(qT[:, tsl], st["t1q"], t2q)
                    mm_half("v", wv_sb, 0)

                def v2():
                    mm_half("v", wv_sb, 1)
                    t2k = ropep.tile([P, 512], b16, tag="t2_k", name="t2")
                    nc.vector.tensor_tensor(t2k, st["sw2"][:, 1],
                                            sin_sb[:, tsl], MUL)
                    nc.vector.tensor_add(kT[:, tsl], st["t1k"], t2k)
                    # v: psum->sbuf copy, XBAR dma-transpose to a dense
                    # [t, d] tile (strided destinations come out scrambled),
                    # then two narrow copies into vaug around the ones column
                    vt = ropep.tile([P, 512], b16, tag="vt", name="vt")
                    copy(vt, st["v"])
                    dense = ropep.tile([P, 4, P], b16, tag="vdense",
                                       name="vdense")
                    nc.sync.dma_start_transpose(dense, vt[:])
                    g0 = tch * 4
                    nc.vector.tensor_copy(vaug[:, g0:g0 + 4, 0:D],
                                          dense[:, :, 0:D])
                    nc.vector.tensor_copy(vaug[:, g0:g0 + 4, D + 1:2 * D + 1],
                                          dense[:, :, D:2 * D])

                return [q1, q2, k1, k2, v1, v2]

            def qkv_chunk(tch):
                for f in qkv_parts(tch, nc.scalar):
                    f()


            def oproj_fillers(iw, tail=False, norm=None):
                """o-projection (rows x Wo^T) for four 128-row blocks, one
                batched output DMA per block. At the tail (last window), each
                closure also carries its own 128-col slice of the normalize
                multiplies so the final chain is as short as possible."""
                outs = []
                for tb in range(4 * iw, 4 * iw + 4):
                    def one(tb=tb):
                        if norm is not None:
                            psyE, recbs = norm
                            off = tb * P - iw * 512
                            for h in range(HPC):
                                nc.vector.tensor_tensor(
                                    yT[D * h:D * h + D, tb * P:(tb + 1) * P],
                                    psyE[h][0:D, off:off + P],
                                    recbs[h][:, off:off + P], MUL)
                        ob2 = attp.tile([P, 2, 512], b16, tag="ob",
                                        name="ob")
                        for mc in range(C // 512):
                            if tail and mc == 1:
                                # score double-buffers are free at the tail
                                pso = psp.tile([P, 512], f32, tag="pss2",
                                               bufs=2, name="pso2")
                            else:
                                pso = scratch()
                            nc.tensor.matmul(
                                pso, yT[:, tb * P:(tb + 1) * P],
                                wo_sb[:, mc * 512:(mc + 1) * 512],
                                start=True, stop=True,
                            )
                            if tail and mc == 1:
                                nc.scalar.copy(ob2[:, mc], pso)
                            else:
                                nc.vector.tensor_copy(ob2[:, mc], pso)
                        nc.sync.dma_start(out_d[tb * P:(tb + 1) * P, :], ob2)
                    outs.append(one)
                return outs

            def win512(iw, parts=(), oproj=(), last=False, head=()):
                """512-wide attention window over i in [512iw, 512iw+512),
                one-jc software-pipelined, with ONE merged exp per jc
                covering both heads ([128, 2x512]). Fillers run in the PE
                slack of the ACT-bound jc loop: qkv chunk micro-parts from
                jc>=1 (always ready), the previous window's o-projection
                from jc>=3 (needs its normalize to have drained). Returns
                this window's o-projection closures for the next window."""
                isl = slice(iw * 512, (iw + 1) * 512)
                njc = 4 * iw + 4
                parts = list(parts)
                oproj = list(oproj)
                psyE = {}
                for h in range(HPC):
                    psyE[h] = psp.tile([D + 1, 512], f32, tag=f"psyE{h}",
                                       name="psyE")

                def emit_attv(p):
                    jc0, a2 = p
                    for h in range(HPC):
                        va = vaug[:, jc0, (D + 1) * h:(D + 1) * h + D + 1]
                        nc.tensor.matmul(psyE[h], va, a2[:, h],
                                         start=(jc0 == 0),
                                         stop=(jc0 == njc - 1))

                def score_exp(jc, isl_):
                    jsl = slice(jc * P, (jc + 1) * P)
                    ps2 = psp.tile([P, 2, 512], f32, tag="pss2", bufs=2,
                                   name="ps2")
                    for h in range(HPC):
                        hb = D * h
                        nc.tensor.matmul(ps2[:, h], kT[hb:hb + D, jsl],
                                         qT[hb:hb + D, isl_],
                                         start=True, stop=True)
                    att2 = attp.tile([P, 2, 512], b16, tag="att2",
                                     name="att2", bufs=6)
                    nc.scalar.activation(att2, ps2, EXP, scale=0.125)
                    return att2

                pend = list(head)
                for jc in range(len(head), njc):
                    att2 = score_exp(jc, isl)
                    if jc >= 4 * iw:
                        m = jc - 4 * iw
                        for h in range(HPC):
                            nc.vector.tensor_tensor(att2[:, h], att2[:, h],
                                                    msk_sb[:, m], MUL)
                    # 2-deep attv pipeline: the first att@v lands at jc2, by
                    # which time the previous window's normalize (the WAR on
                    # this window's psyE accumulators) has drained
                    if len(pend) == 3:
                        emit_attv(pend.pop(0))
                    pend.append((jc, att2))
                    # keep the last 3 jc free of fillers: their DVE work
                    # (ob / dense copies) would delay the diagonal masks the
                    # end-of-window attv flush waits on
                    if jc < njc - 3:
                        if jc >= 1 and parts:
                            parts.pop(0)()
                        elif jc >= 3 and oproj and (
                                jc % 2 == 1
                                or 2 * len(oproj) > njc - 3 - jc):
                            oproj.pop(0)()
                nxt = []
                # iw=0 can't pre-compute w1's scores: chunk 1's rope (qT
                # cols 512:1024) is still in this window's leftover fillers
                if not last and iw > 0:
                    # pre-compute the next window's first two scores/exps so
                    # PE has work while this window's last exp+mask drain
                    isl_n = slice((iw + 1) * 512, (iw + 2) * 512)
                    for jc_n in range(2):
                        nxt.append((jc_n, score_exp(jc_n, isl_n)))
                for p in pend:
                    emit_attv(p)
                recs, recbs = {}, {}
                for h in range(HPC):
                    recs[h] = smallp.tile([1, 512], f32, tag=f"rec{h}",
                                          name="rec")
                    nc.vector.reciprocal(recs[h], psyE[h][D:D + 1, :])
                for h in range(HPC):
                    recbs[h] = smallp.tile([D, 512], f32, tag=f"recb{h}",
                                           name="recb")
                    nc.gpsimd.partition_broadcast(recbs[h], recs[h])
                if last:
                    for f in parts + oproj:
                        f()
                    return oproj_fillers(iw, tail=True,
                                         norm=(psyE, recbs)), ()
                for h in range(HPC):
                    nc.vector.tensor_tensor(yT[D * h:D * h + D, isl],
                                            psyE[h][0:D, :], recbs[h], MUL)
                for f in parts + oproj:   # leftovers: after the norm so its
                    f()                   # chain drains under their PE work
                return oproj_fillers(iw), nxt

            qkv_chunk(0)
            c1 = qkv_parts(1, nc.scalar)
            for f in c1[:4]:
                f()
            late_consts()
            op0, hd = win512(0, parts=c1[4:] + qkv_parts(2, nc.scalar))
            op1, hd = win512(1, parts=qkv_parts(3, nc.scalar), head=hd)
            op2, hd = win512(2, parts=qkv_parts(4, nc.scalar), oproj=op0,
                             head=hd)
            op3, hd = win512(3, parts=qkv_parts(5, nc.vector), oproj=op1,
                             head=hd)
            op4, hd = win512(4, parts=qkv_parts(6, nc.vector), oproj=op2,
                             head=hd)
            op5, hd = win512(5, parts=qkv_parts(7, nc.vector), oproj=op3,
                             head=hd)
            op6, hd = win512(6, oproj=op4 + op5, head=hd)
            op7, _ = win512(7, oproj=op6, last=True, head=hd)
            for f in op7:
                f()

    nc.compile()
    return nc


def _perm_deinterleave():
    """Row permutation for Wq/Wk: per head, even rows then odd rows."""
    perm = []
    for h in range(H):
        base = h * D
        perm += [base + 2 * k for k in range(D // 2)]
        perm += [base + 2 * k + 1 for k in range(D // 2)]
    return np.array(perm)


def make_core_inputs(x, freqs_cos, freqs_sin, Wq, Wk, Wv, Wo, t=T):
    """Host-side sharding/layout prep. Returns per-core input dicts."""
    x = np.asarray(x, np.float32).reshape(t, C)
    fc = np.asarray(freqs_cos, np.float32)
    fs = np.asarray(freqs_sin, np.float32)
    Wq = np.asarray(Wq, np.float32)
    Wk = np.asarray(Wk, np.float32)
    Wv = np.asarray(Wv, np.float32)
    Wo = np.asarray(Wo, np.float32)

    xt = np.ascontiguousarray(x.T).astype(bf16)                  # [C, t]
    perm = _perm_deinterleave()
    Wq_p, Wk_p = Wq[perm], Wk[perm]

    # rope factor tables in the de-interleaved [dd, t] layout
    kidx = np.arange(P) % 32
    sgn = np.where((np.arange(P) // 32) % 2 == 0, -1.0, 1.0).astype(np.float32)
    cosb = fc.T[kidx].astype(bf16)                               # [128, t]
    sinb = (fs.T[kidx] * sgn[:, None]).astype(bf16)

    # diagonal-tile causal masks: mask4[j, m, i] = 1 iff 128*m + j <= i
    jj = np.arange(P)[:, None, None]
    mm = np.arange(4)[None, :, None]
    ii = np.arange(512)[None, None, :]
    mask4 = ((P * mm + jj) <= ii).astype(bf16)

    def pack_w(w):
        # [C, DD] -> [P, C//P, DD]: one contiguous 2KB run per partition
        return np.ascontiguousarray(
            w.T.reshape(C // P, P, DD).transpose(1, 0, 2)).astype(bf16)

    in_maps = []
    for c in range(NCORES):
        rows = slice(c * DD, (c + 1) * DD)
        in_maps.append({
            "xt": xt,
            "wq": pack_w(Wq_p[rows]),
            "wk": pack_w(Wk_p[rows]),
            "wv": pack_w(Wv[rows]),
            "wo": np.ascontiguousarray(Wo[:, rows].T).astype(bf16),
            "cosb": cosb,
            "sinb": sinb,
            "mask4": mask4,
        })
    return in_maps


def run(inputs, trace=False):
    """Compile once, run on 8 cores, host-sum partials. Returns (out, results)."""
    import sys
    if "/opt/trn_rl_repo" not in sys.path:
        sys.path.insert(0, "/opt/trn_rl_repo")
    from concourse.bass_utils import run_bass_kernel_spmd

    if "nc" not in _nc_cache:
        _nc_cache["nc"] = _build_nc()
    nc = _nc_cache["nc"]

    in_maps = make_core_inputs(**inputs)
    res = run_bass_kernel_spmd(nc, in_maps, core_ids=list(range(NCORES)),
                               trace=trace)
    out = np.zeros((T, C), np.float64)
    for r in res.results:
        out += r["opart"].astype(np.float64)
    return out.astype(np.float32).reshape(1, T, C), res


def kernel(**inputs):
    import sys
    if "/opt/trn_rl_repo" not in sys.path:
        sys.path.insert(0, "/opt/trn_rl_repo")
    out, _ = run(inputs)
    return out



# revision 55
# speedup vs baseline: 1.3678x; 1.0202x over previous
"""Causal self-attention with RoPE, tensor-parallel over heads on 8 trn2 cores.

Reference computation (B=1, T=4096, C=1024, h=16, d=64, fp32):
    q/k/v = x @ W{q,k,v}^T ; rope(q), rope(k) ; causal softmax(q k^T / 8) v ; @ Wo^T

Sharding: 2 heads per core (tensor parallel). Each core reads the full x
(transposed + bf16 on host) and its slice of Wq/Wk/Wv (column-parallel) and
Wo (row-parallel). Cores emit partial o-projections; the host sums them.

Device-side layout choices:
  - qT/kT [dhead(=128 both heads) x T] with the head dim de-interleaved
    (rope real parts in partitions 0-31 / 64-95, imag in 32-63 / 96-127) so
    rope's pair-swap is a partition-block swap done by 4 small DMAs.
  - scores are computed transposed: sT[j, i] = sum_d kT[d,j] qT[d,i], so the
    softmax normalizer is a sum over PARTITIONS, obtained for free by
    augmenting v with a ones column in the att @ v matmul (row 64 of the
    y-psum accumulates the denominator).
  - v is produced transposed like q/k then PE-transposed to natural [t, d]
    blocks (needed as the stationary operand of the att@v matmul).
  - causal masking: only diagonal j-tiles need masking; 4 static [128,512]
    masks (one per 128-offset within a 512 column group) multiply exp'd
    scores. Fully-masked subtiles are skipped in the att@v accumulation.
"""

import numpy as np
import ml_dtypes

bf16 = ml_dtypes.bfloat16

T, C, H, D = 4096, 1024, 16, 64
NCORES = 8
HPC = H // NCORES          # heads per core
DD = HPC * D               # per-core qkv features (=128)
P = 128

_nc_cache = {}


def _build_nc(t=T):
    import concourse.bass as bass
    import concourse.tile as tile
    import concourse.mybir as mybir
    from concourse import bacc
    from concourse.masks import make_identity

    f32 = mybir.dt.float32
    b16 = mybir.dt.bfloat16
    MUL = mybir.AluOpType.mult
    EXP = mybir.ActivationFunctionType.Exp

    nt = t // 512            # qkv t-chunks
    nw = t // 1024           # attention query windows
    njb = t // P             # key blocks

    nc = bacc.Bacc("TRN2")

    xt_d = nc.dram_tensor("xt", [C, t], b16, kind="ExternalInput")
    # weights are host-prepacked to [P, C//P, DD] so the load is one
    # contiguous 2KB run per partition (full-rate DMA descriptors)
    wq_d = nc.dram_tensor("wq", [P, C // P, DD], b16, kind="ExternalInput")
    wk_d = nc.dram_tensor("wk", [P, C // P, DD], b16, kind="ExternalInput")
    wv_d = nc.dram_tensor("wv", [P, C // P, DD], b16, kind="ExternalInput")
    wo_d = nc.dram_tensor("wo", [DD, C], b16, kind="ExternalInput")
    cos_d = nc.dram_tensor("cosb", [P, t], b16, kind="ExternalInput")
    sin_d = nc.dram_tensor("sinb", [P, t], b16, kind="ExternalInput")
    msk_d = nc.dram_tensor("mask4", [P, 4, 512], b16, kind="ExternalInput")
    out_d = nc.dram_tensor("opart", [t, C], b16, kind="ExternalOutput")

    with tile.TileContext(nc) as tc:
        with (
            tc.tile_pool(name="const", bufs=1) as constp,
            tc.tile_pool(name="xload", bufs=3) as xload,
            tc.tile_pool(name="rope", bufs=3) as ropep,
            tc.tile_pool(name="att", bufs=4) as attp,
            tc.tile_pool(name="small", bufs=4) as smallp,
            tc.tile_pool(name="ps", bufs=1, space="PSUM") as psp,
        ):
            # ---- constants / persistent tensors. DMA issue order is chosen
            # so the first qkv matmuls (need wq + x chunk 0) start ASAP:
            # wq -> xt0 -> wk/wv -> cos/sin (needed at first rope) -> rest.
            xt_pre = {}

            def load_xt(tch):
                tsl = slice(tch * 512, (tch + 1) * 512)
                xt = xload.tile([P, C // P, 512], b16, name="xt")
                nc.sync.dma_start(
                    xt, xt_d[:].rearrange("(co p) t -> p co t", p=P)[:, :, tsl]
                )
                xt_pre[tch] = xt

            wq_sb = constp.tile([P, C // P, DD], b16)
            nc.sync.dma_start(wq_sb, wq_d[:])
            # x chunk 0 in four slices so the first qkv matmuls start sooner
            xt0 = xload.tile([P, C // P, 512], b16, name="xt0")
            xt_view = xt_d[:].rearrange("(co p) t -> p co t", p=P)
            for c4 in range(4):
                nc.sync.dma_start(xt0[:, 2 * c4:2 * c4 + 2],
                                  xt_view[:, 2 * c4:2 * c4 + 2, 0:512])
            xt_pre[0] = xt0
            wk_sb = constp.tile([P, C // P, DD], b16)
            nc.sync.dma_start(wk_sb, wk_d[:])
            load_xt(1)
            wv_sb = constp.tile([P, C // P, DD], b16)
            nc.sync.dma_start(wv_sb, wv_d[:])
            # rope tables: only the first two chunks' columns now — the big
            # tails load after chunk 0/1's swap DMAs so the first rope isn't
            # stuck behind them in the transfer queue
            cos_sb = constp.tile([P, t], b16)
            nc.sync.dma_start(cos_sb[:, 0:1024], cos_d[:, 0:1024])
            sin_sb = constp.tile([P, t], b16)
            nc.sync.dma_start(sin_sb[:, 0:1024], sin_d[:, 0:1024])
            wo_sb = constp.tile([DD, C], b16)
            msk_sb = constp.tile([P, 4, 512], b16)

            def late_consts():
                nc.sync.dma_start(msk_sb, msk_d[:])
                nc.sync.dma_start(cos_sb[:, 1024:], cos_d[:, 1024:])
                nc.sync.dma_start(sin_sb[:, 1024:], sin_d[:, 1024:])
                nc.sync.dma_start(wo_sb, wo_d[:])

            qT = constp.tile([P, t], b16)   # rope'd q, both heads
            kT = constp.tile([P, t], b16)
            yT = constp.tile([P, t], b16)   # normalized attention output
            # v in natural layout per 128-block, +ones cols at 64 and 129
            vaug = constp.tile([P, njb, 2 * D + 2], b16)
            nc.vector.memset(vaug[:, :, D], 1.0)
            nc.vector.memset(vaug[:, :, 2 * D + 1], 1.0)

            # PSUM budget (8 banks): scratch 2KB x2 (qkv psums + o-proj) +
            # pss2 4KB x2 (scores, double-buffered) + psyE{h} 2KB x2 (y+denom
            # accumulators) = 16KB.
            def scratch():
                return psp.tile([P, 512], f32, tag="scratch", bufs=2,
                                name="scr")

            def qkv_parts(tch, cp):
                """Six micro-closures (~0.85us of PE each) so a chunk's
                matmuls can be spread one-per-jc through an attention
                window's PE slack. cp = engine for the psum->sbuf copies
                (ACT before the windows saturate it, DVE after)."""
                tsl = slice(tch * 512, (tch + 1) * 512)
                st = {}

                def copy(dst, src_):
                    if cp is nc.scalar:
                        nc.scalar.copy(dst, src_)
                    else:
                        cp.tensor_copy(dst, src_)

                def mm_half(name, w_sb, half):
                    if name == "q" and half == 0:
                        st["xt"] = xt_pre.pop(tch)
                        st["q"] = scratch()
                    if name == "k" and half == 0:
                        if tch + 2 < nt and tch + 2 not in xt_pre:
                            load_xt(tch + 2)
                    if half == 0 and name != "q":
                        st[name] = scratch()
                    ps = st[name]
                    for ci in range(4 * half, 4 * half + 4):
                        nc.tensor.matmul(
                            ps, w_sb[:, ci], st["xt"][:, ci],
                            start=(ci == 0), stop=(ci == C // P - 1),
                        )

                def q1():
                    mm_half("q", wq_sb, 0)

                def q2():
                    mm_half("q", wq_sb, 1)
                    # q and k share one [128, 2, 512] tile so the rope
                    # pair-swap is 4 partition-block DMAs covering both
                    st["qf2"] = ropep.tile([P, 2, 512], b16, tag="qf2",
                                           name="qf2")
                    copy(st["qf2"][:, 0], st["q"])

                def k1():
                    mm_half("k", wk_sb, 0)

                def k2():
                    mm_half("k", wk_sb, 1)
                    qf2 = st["qf2"]
                    copy(qf2[:, 1], st["k"])
                    sw2 = ropep.tile([P, 2, 512], b16, tag="sw2", name="sw2")
                    nc.sync.dma_start(sw2[0:32], qf2[32:64])
                    nc.sync.dma_start(sw2[32:64], qf2[0:32])
                    nc.sync.dma_start(sw2[64:96], qf2[96:128])
                    nc.sync.dma_start(sw2[96:128], qf2[64:96])
                    st["sw2"] = sw2
                    for i, name in enumerate(("q", "k")):
                        tl = ropep.tile([P, 512], b16, tag=f"t1_{name}",
                                        name="t1")
                        nc.vector.tensor_tensor(tl, qf2[:, i],
                                                cos_sb[:, tsl], MUL)
                        st[f"t1{name}"] = tl

                def v1():
                    t2q = ropep.tile([P, 512], b16, tag="t2_q", name="t2")
                    nc.vector.tensor_tensor(t2q, st["sw2"][:, 0],
                                            sin_sb[:, tsl], MUL)
                    nc.vector.tensor_add(qT[:, tsl], st["t1q"], t2q)
                    mm_half("v", wv_sb, 0)

                def v2():
                    mm_half("v", wv_sb, 1)
                    t2k = ropep.tile([P, 512], b16, tag="t2_k", name="t2")
                    nc.vector.tensor_tensor(t2k, st["sw2"][:, 1],
                                            sin_sb[:, tsl], MUL)
                    nc.vector.tensor_add(kT[:, tsl], st["t1k"], t2k)
                    # v: psum->sbuf copy, XBAR dma-transpose to a dense
                    # [t, d] tile (strided destinations come out scrambled),
                    # then two narrow copies into vaug around the ones column
                    vt = ropep.tile([P, 512], b16, tag="vt", name="vt")
                    copy(vt, st["v"])
                    dense = ropep.tile([P, 4, P], b16, tag="vdense",
                                       name="vdense")
                    nc.sync.dma_start_transpose(dense, vt[:])
                    g0 = tch * 4
                    # GPSIMD (SBUF->SBUF): keeps the XBAR-DMA latency out of
                    # DVE's in-order queue, which the diagonal masks share
                    nc.gpsimd.tensor_copy(vaug[:, g0:g0 + 4, 0:D],
                                          dense[:, :, 0:D])
                    nc.gpsimd.tensor_copy(vaug[:, g0:g0 + 4, D + 1:2 * D + 1],
                                          dense[:, :, D:2 * D])

                return [q1, q2, k1, k2, v1, v2]

            def qkv_chunk(tch):
                for f in qkv_parts(tch, nc.scalar):
                    f()


            def oproj_fillers(iw, tail=False, norm=None):
                """o-projection (rows x Wo^T) for four 128-row blocks, one
                batched output DMA per block. At the tail (last window), each
                closure also carries its own 128-col slice of the normalize
                multiplies so the final chain is as short as possible."""
                outs = []
                for tb in range(4 * iw, 4 * iw + 4):
                    def one(tb=tb):
                        if norm is not None:
                            psyE, recbs = norm
                            off = tb * P - iw * 512
                            for h in range(HPC):
                                nc.vector.tensor_tensor(
                                    yT[D * h:D * h + D, tb * P:(tb + 1) * P],
                                    psyE[h][0:D, off:off + P],
                                    recbs[h][:, off:off + P], MUL)
                        ob2 = attp.tile([P, 2, 512], b16, tag="ob",
                                        name="ob")
                        for mc in range(C // 512):
                            if tail and mc == 1:
                                # score double-buffers are free at the tail
                                pso = psp.tile([P, 512], f32, tag="pss2",
                                               bufs=2, name="pso2")
                            else:
                                pso = scratch()
                            nc.tensor.matmul(
                                pso, yT[:, tb * P:(tb + 1) * P],
                                wo_sb[:, mc * 512:(mc + 1) * 512],
                                start=True, stop=True,
                            )
                            if tail and mc == 1:
                                nc.scalar.copy(ob2[:, mc], pso)
                            else:
                                nc.vector.tensor_copy(ob2[:, mc], pso)
                        nc.sync.dma_start(out_d[tb * P:(tb + 1) * P, :], ob2)
                    outs.append(one)
                return outs

            def win512(iw, parts=(), oproj=(), last=False, head=()):
                """512-wide attention window over i in [512iw, 512iw+512),
                one-jc software-pipelined, with ONE merged exp per jc
                covering both heads ([128, 2x512]). Fillers run in the PE
                slack of the ACT-bound jc loop: qkv chunk micro-parts from
                jc>=1 (always ready), the previous window's o-projection
                from jc>=3 (needs its normalize to have drained). Returns
                this window's o-projection closures for the next window."""
                isl = slice(iw * 512, (iw + 1) * 512)
                njc = 4 * iw + 4
                parts = list(parts)
                oproj = list(oproj)
                psyE = {}
                for h in range(HPC):
                    psyE[h] = psp.tile([D + 1, 512], f32, tag=f"psyE{h}",
                                       name="psyE")

                def emit_attv(p):
                    jc0, a2 = p
                    for h in range(HPC):
                        va = vaug[:, jc0, (D + 1) * h:(D + 1) * h + D + 1]
                        nc.tensor.matmul(psyE[h], va, a2[:, h],
                                         start=(jc0 == 0),
                                         stop=(jc0 == njc - 1))

                def score_exp(jc, isl_):
                    jsl = slice(jc * P, (jc + 1) * P)
                    ps2 = psp.tile([P, 2, 512], f32, tag="pss2", bufs=2,
                                   name="ps2")
                    for h in range(HPC):
                        hb = D * h
                        nc.tensor.matmul(ps2[:, h], kT[hb:hb + D, jsl],
                                         qT[hb:hb + D, isl_],
                                         start=True, stop=True)
                    att2 = attp.tile([P, 2, 512], b16, tag="att2",
                                     name="att2", bufs=6)
                    nc.scalar.activation(att2, ps2, EXP, scale=0.125)
                    return att2

                pend = list(head)
                for jc in range(len(head), njc):
                    att2 = score_exp(jc, isl)
                    if jc >= 4 * iw:
                        m = jc - 4 * iw
                        for h in range(HPC):
                            nc.vector.tensor_tensor(att2[:, h], att2[:, h],
                                                    msk_sb[:, m], MUL)
                    # 2-deep attv pipeline: the first att@v lands at jc2, by
                    # which time the previous window's normalize (the WAR on
                    # this window's psyE accumulators) has drained
                    if len(pend) == 3:
                        emit_attv(pend.pop(0))
                    pend.append((jc, att2))
                    # keep the last 3 jc free of fillers: their DVE work
                    # (ob / dense copies) would delay the diagonal masks the
                    # end-of-window attv flush waits on
                    if jc < njc - 3:
                        if jc >= 1 and parts:
                            parts.pop(0)()
                        elif jc >= 3 and oproj and (
                                jc % 2 == 1
                                or 2 * len(oproj) > njc - 3 - jc):
                            oproj.pop(0)()
                nxt = []
                # iw=0 can't pre-compute w1's scores: chunk 1's rope (qT
                # cols 512:1024) is still in this window's leftover fillers
                if not last and iw > 0:
                    # pre-compute the next window's first two scores/exps so
                    # PE has work while this window's last exp+mask drain
                    isl_n = slice((iw + 1) * 512, (iw + 2) * 512)
                    for jc_n in range(2):
                        nxt.append((jc_n, score_exp(jc_n, isl_n)))
                for p in pend:
                    emit_attv(p)
                recs, recbs = {}, {}
                for h in range(HPC):
                    recs[h] = smallp.tile([1, 512], f32, tag=f"rec{h}",
                                          name="rec")
                    nc.vector.reciprocal(recs[h], psyE[h][D:D + 1, :])
                for h in range(HPC):
                    recbs[h] = smallp.tile([D, 512], f32, tag=f"recb{h}",
                                           name="recb")
                    nc.gpsimd.partition_broadcast(recbs[h], recs[h])
                if last:
                    for f in parts + oproj:
                        f()
                    return oproj_fillers(iw, tail=True,
                                         norm=(psyE, recbs)), ()
                for h in range(HPC):
                    nc.vector.tensor_tensor(yT[D * h:D * h + D, isl],
                                            psyE[h][0:D, :], recbs[h], MUL)
                for f in parts + oproj:   # leftovers: after the norm so its
                    f()                   # chain drains under their PE work
                return oproj_fillers(iw), nxt

            qkv_chunk(0)
            c1 = qkv_parts(1, nc.scalar)
            for f in c1[:4]:
                f()
            late_consts()
            op0, hd = win512(0, parts=c1[4:] + qkv_parts(2, nc.scalar))
            op1, hd = win512(1, parts=qkv_parts(3, nc.scalar), head=hd)
            op2, hd = win512(2, parts=qkv_parts(4, nc.scalar), oproj=op0,
                             head=hd)
            op3, hd = win512(3, parts=qkv_parts(5, nc.vector), oproj=op1,
                             head=hd)
            op4, hd = win512(4, parts=qkv_parts(6, nc.vector), oproj=op2,
                             head=hd)
            op5, hd = win512(5, parts=qkv_parts(7, nc.vector), oproj=op3,
                             head=hd)
            op6, hd = win512(6, oproj=op4 + op5, head=hd)
            op7, _ = win512(7, oproj=op6, last=True, head=hd)
            for f in op7:
                f()

    nc.compile()
    return nc


def _perm_deinterleave():
    """Row permutation for Wq/Wk: per head, even rows then odd rows."""
    perm = []
    for h in range(H):
        base = h * D
        perm += [base + 2 * k for k in range(D // 2)]
        perm += [base + 2 * k + 1 for k in range(D // 2)]
    return np.array(perm)


def make_core_inputs(x, freqs_cos, freqs_sin, Wq, Wk, Wv, Wo, t=T):
    """Host-side sharding/layout prep. Returns per-core input dicts."""
    x = np.asarray(x, np.float32).reshape(t, C)
    fc = np.asarray(freqs_cos, np.float32)
    fs = np.asarray(freqs_sin, np.float32)
    Wq = np.asarray(Wq, np.float32)
    Wk = np.asarray(Wk, np.float32)
    Wv = np.asarray(Wv, np.float32)
    Wo = np.asarray(Wo, np.float32)

    xt = np.ascontiguousarray(x.T).astype(bf16)                  # [C, t]
    perm = _perm_deinterleave()
    Wq_p, Wk_p = Wq[perm], Wk[perm]

    # rope factor tables in the de-interleaved [dd, t] layout
    kidx = np.arange(P) % 32
    sgn = np.where((np.arange(P) // 32) % 2 == 0, -1.0, 1.0).astype(np.float32)
    cosb = fc.T[kidx].astype(bf16)                               # [128, t]
    sinb = (fs.T[kidx] * sgn[:, None]).astype(bf16)

    # diagonal-tile causal masks: mask4[j, m, i] = 1 iff 128*m + j <= i
    jj = np.arange(P)[:, None, None]
    mm = np.arange(4)[None, :, None]
    ii = np.arange(512)[None, None, :]
    mask4 = ((P * mm + jj) <= ii).astype(bf16)

    def pack_w(w):
        # [C, DD] -> [P, C//P, DD]: one contiguous 2KB run per partition
        return np.ascontiguousarray(
            w.T.reshape(C // P, P, DD).transpose(1, 0, 2)).astype(bf16)

    in_maps = []
    for c in range(NCORES):
        rows = slice(c * DD, (c + 1) * DD)
        in_maps.append({
            "xt": xt,
            "wq": pack_w(Wq_p[rows]),
            "wk": pack_w(Wk_p[rows]),
            "wv": pack_w(Wv[rows]),
            "wo": np.ascontiguousarray(Wo[:, rows].T).astype(bf16),
            "cosb": cosb,
            "sinb": sinb,
            "mask4": mask4,
        })
    return in_maps


def run(inputs, trace=False):
    """Compile once, run on 8 cores, host-sum partials. Returns (out, results)."""
    import sys
    if "/opt/trn_rl_repo" not in sys.path:
        sys.path.insert(0, "/opt/trn_rl_repo")
    from concourse.bass_utils import run_bass_kernel_spmd

    if "nc" not in _nc_cache:
        _nc_cache["nc"] = _build_nc()
    nc = _nc_cache["nc"]

    in_maps = make_core_inputs(**inputs)
    res = run_bass_kernel_spmd(nc, in_maps, core_ids=list(range(NCORES)),
                               trace=trace)
    out = np.zeros((T, C), np.float64)
    for r in res.results:
        out += r["opart"].astype(np.float64)
    return out.astype(np.float32).reshape(1, T, C), res


def kernel(**inputs):
    import sys
    if "/opt/trn_rl_repo" not in sys.path:
        sys.path.insert(0, "/opt/trn_rl_repo")
    out, _ = run(inputs)
    return out

